# revision 1
# baseline (speedup 1.0000x reference)
"""DFT-D3 (zero damping, static all-pairs) two-body dispersion energy on 8
Trainium2 NeuronCores.

Strategy (matches the i-slab sharding hint):
  - Each core owns a slab of 64 atoms i; it computes the [64, 512, 27]
    pair-geometry tensor r2 with ONE bf16 TensorE matmul per 512-column
    chunk (3-way bf16-split operands give ~24-bit accuracy for
    |p_j|^2 - 2 p_j.y_is + |y_is|^2; per-core self-pair offset rows are
    merged into the same matmul).  All per-element rationals run in log
    space: L2 = ln r2, r^-k = exp(-k/2 L2), and every damping factor
    1/(1 + c r^-k) = sigmoid(k/2 L2 - ln c) -- so the Vector engine only
    does cheap tensor ops while Ln/Exp/Sigmoid run on the Scalar engine
    (activation table loads batched: ln/exp set, then sigmoid set, then
    the phi exponentials).  Reduced over shifts s -> A_ij, B_ij slabs and
    CN partial sums.
  - Per-atom CN of the slab is AllGather'ed across the 8 cores on-device.
  - The c6(cn_i, cn_j) interpolation uses the separable structure of the
    c6ab reference grids (cn_i grid varies only along a, cn_j only along b,
    all c6ref > 0), collapsing phase 2 to per-atom 5-vectors phi plus small
    TensorE contractions E = sum_ij G_ij * (P C Q^T)_ij. Host verifies the
    separability; a numpy fallback handles pathological inputs.

The whole computation is a single NEFF launch per call; inputs are
reformatted host-side into matmul operands (one-hot matrices, slab tables).
"""
import os
os.environ.setdefault("JAX_PLATFORMS", "cpu")

import numpy as np
import ml_dtypes

import concourse.bass as bass
import concourse.mybir as mybir
from concourse.tile import TileContext, add_dep_helper
from concourse.bass_utils import run_bass_kernel_spmd

F32 = mybir.dt.float32
AF = mybir.ActivationFunctionType
OP = mybir.AluOpType

# D3 constants
AUTOANG = 0.52917726
AUTOEV = 27.21138505
K1, K3 = 16.0, -4.0
CUTOFF, CNTHR = 95.0, 40.0
S6, RS6, S18, RS18, ALP = 1.0, 1.217, 0.722, 1.0, 14.0

N = 512          # atoms
NS = 27          # lattice shifts
NCORES = 8
SLAB = N // NCORES           # 64 atoms per core
JB = 4                       # j blocks of 128
FREE = SLAB * NS             # 1728
NZ = 95                      # species table size
NG = 5                       # cn grid points per axis

R2MIN = 1.43e-3              # clamp: below this every quantity saturates
SELF_R2 = 1.0e4              # value added to self pairs via matmul
import math
C14L = float(math.log(6.0) + 14.0 * math.log(RS6))   # ln(6*RS6^14)
C16L = float(math.log(6.0) + 16.0 * math.log(RS18))  # ln(6*RS18^16)
CB8 = float(3.0 * S18)

_CHUNKS = [(0, 512), (512, 1024), (1024, 1536), (1536, 1728)]


def _bc_s(ap2d, s=NS):
    """[128, M] AP -> [128, M, s] with stride-0 broadcast over s."""
    a3 = ap2d[:, :, None]
    new = [list(a3.ap[0]), list(a3.ap[1]), [0, s]]
    return bass.AP(a3.tensor, a3.offset, new)


def _split_excess_waits(nc, max_waits=1):
    """This walrus build accepts at most one sync wait per instruction;
    Tile's tail drain can carry several. Hoist excess waits onto inserted
    drains on the same engine (sequential waits == conjunction)."""
    n_split = 0
    for f in nc.m.functions:
        for b in f.blocks:
            new_list = []
            changed = False
            for ins in b.instructions:
                si = ins.sync_info
                if si is not None:
                    waits = list(si.on_wait or [])
                    updates = list(si.on_update or [])
                    if len(waits) > max_waits:
                        excess = waits[: len(waits) - max_waits]
                        keep = waits[len(waits) - max_waits:]
                        for w in excess:
                            d = mybir.InstDrain(
                                name=f"I-waitsplit-{n_split}", ins=[], outs=[])
                            n_split += 1
                            d.engine = ins.engine
                            d.sync_info = mybir.SyncInfo(on_wait=[w], on_update=[])
                            new_list.append(d)
                            changed = True
                        ins.sync_info = mybir.SyncInfo(
                            on_wait=list(keep), on_update=list(updates))
                new_list.append(ins)
            if changed:
                b.instructions = new_list
    return n_split


_orig_clear_sems = bass.Bass.clear_and_free_semaphores


def _chunked_clear_sems(self, sems, _chunk=4):
    """This walrus build rejects EVENT_SEMAPHORE_RANGE_CLEAR over wide
    ranges; clear in chunks of <=4 (the size Tile emits for tiny kernels,
    which compiles fine)."""
    nums = sorted(s.num if hasattr(s, "num") else s for s in sems)
    for i in range(0, len(nums), _chunk):
        _orig_clear_sems(self, nums[i:i + _chunk])


bass.Bass.clear_and_free_semaphores = _chunked_clear_sems


def build_program():
    nc = bass.Bass(num_devices=NCORES)

    def din(name, shape):
        return nc.dram_tensor(name, shape, F32, kind="ExternalInput")

    BF16 = mybir.dt.bfloat16
    # shared inputs (same array on every core)
    Lcat = nc.dram_tensor("Lcat", [94, N], BF16, kind="ExternalInput")
    Lrco = din("Lrco", [2, N])
    r0abT = din("r0abT", [NZ, NZ])
    ohZT = din("ohZT", [NZ, N])
    ohZ = din("ohZ", [N, NZ])
    r2r4c = din("r2r4c", [128, JB])
    C2h = nc.dram_tensor("C2h", [NZ * NG, NZ * NG], BF16, kind="ExternalInput")
    C2l = nc.dram_tensor("C2l", [NZ * NG, NZ * NG], BF16, kind="ExternalInput")
    gneg = din("gneg", [128, NG])
    # per-core inputs
    Rcat = nc.dram_tensor("Rcat", [94, FREE], BF16, kind="ExternalInput")
    Rrco = din("Rrco", [2, SLAB])
    ohZiT = din("ohZiT", [NZ, SLAB])
    r2r4sl = din("r2r4sl", [1, SLAB])

    e_part = nc.dram_tensor("e_part", [SLAB], F32, kind="ExternalOutput")

    with TileContext(nc) as tc:
        with (
            tc.tile_pool(name="const", bufs=1) as cpool,
            tc.tile_pool(name="chain", bufs=6) as bpool,
            tc.tile_pool(name="sm", bufs=3) as spool,
            tc.tile_pool(name="u2p", bufs=1) as u2pool,
            tc.tile_pool(name="red", bufs=1) as rpool,
            tc.tile_pool(name="ph2", bufs=1) as ppool,
            tc.tile_pool(name="psA", bufs=1, space="PSUM") as psA,
            tc.tile_pool(name="psS", bufs=1, space="PSUM") as psS,
            tc.tile_pool(name="psK", bufs=1, space="PSUM") as psK,
            tc.tile_pool(name="dram", bufs=1, space="DRAM") as dpool,
        ):
            # ---------- load constants / operands ----------
            def load(t, shape, tag):
                tile = cpool.tile(shape, F32, tag=tag)
                nc.sync.dma_start(tile[:], t[:])
                return tile

            # phase-1-critical loads first
            Lcat_s = cpool.tile([94, N], mybir.dt.bfloat16, tag="Lcat")
            nc.sync.dma_start(Lcat_s[:], Lcat[:])
            Rcat_s = cpool.tile([94, FREE], mybir.dt.bfloat16, tag="Rcat")
            nc.sync.dma_start(Rcat_s[:], Rcat[:])
            Lrco_s = load(Lrco, [2, N], "Lrco")
            r0abT_s = load(r0abT, [NZ, NZ], "r0abT")
            ohZT_s = load(ohZT, [NZ, N], "ohZT")
            Rrco_s = load(Rrco, [2, SLAB], "Rrco")
            ohZiT_s = load(ohZiT, [NZ, SLAB], "ohZiT")
            r2r4sl_s = load(r2r4sl, [1, SLAB], "r2r4sl")
            r2r4c_s = load(r2r4c, [128, JB], "r2r4c")
            gneg_s = load(gneg, [128, NG], "gneg")
            # phase-2-only loads (overlap with compute)
            ohZ_b = []
            for b in range(JB):
                t = cpool.tile([128, NZ], F32, tag=f"ohZ_{b}")
                nc.sync.dma_start(t[:], ohZ[b * 128:(b + 1) * 128, :])
                ohZ_b.append(t)
            C2h_a, C2l_a = [], []
            for a in range(NG):
                th = cpool.tile([NZ, NZ * NG], mybir.dt.bfloat16, tag=f"C2h_{a}")
                nc.sync.dma_start(th[:], C2h[a * NZ:(a + 1) * NZ, :])
                C2h_a.append(th)
                tl = cpool.tile([NZ, NZ * NG], mybir.dt.bfloat16, tag=f"C2l_{a}")
                nc.sync.dma_start(tl[:], C2l[a * NZ:(a + 1) * NZ, :])
                C2l_a.append(tl)

            ones1x95 = cpool.tile([1, NZ], F32, tag="ones95")
            nc.gpsimd.memset(ones1x95[:], 1.0)
            ones128x1 = cpool.tile([128, 1], F32, tag="ones128")
            nc.gpsimd.memset(ones128x1[:], 1.0)
            ones1x128 = cpool.tile([1, 128], F32, tag="ones1x128")
            nc.gpsimd.memset(ones1x128[:], 1.0)
            negk1 = cpool.tile([128, 1], F32, tag="negk1")
            nc.gpsimd.memset(negk1[:], -K1)

            # ---------- prep matmuls ----------
            # R1[z1, i] = r0ab[z1, Z_i]
            R1_ps = psS.tile([NZ, SLAB], F32, tag="small")
            nc.tensor.matmul(R1_ps[:], r0abT_s[:], ohZiT_s[:], start=True, stop=True)
            R1_s = cpool.tile([NZ, SLAB], F32, tag="R1")
            nc.scalar.copy(R1_s[:], R1_ps[:])

            # r2r4 of slab broadcast down partitions: [128, 64]
            r2r4i_ps = psS.tile([128, SLAB], F32, tag="small")
            nc.tensor.matmul(r2r4i_ps[:], ones1x128[:], r2r4sl_s[:],
                             start=True, stop=True)
            r2r4i_s = cpool.tile([128, SLAB], F32, tag="r2r4i")
            nc.scalar.copy(r2r4i_s[:], r2r4i_ps[:])

            # per block: ln(c14)/ln(c16) pair tables and rco
            lnc14_s, lnc16_s, rco_s = [], [], []
            for b in range(JB):
                jsl = slice(b * 128, (b + 1) * 128)
                r0p_ps = psS.tile([128, SLAB], F32, tag="small")
                nc.tensor.matmul(r0p_ps[:], ohZT_s[:, jsl], R1_s[:],
                                 start=True, stop=True)
                lr = spool.tile([128, SLAB], F32, tag="lnr0")
                nc.scalar.activation(lr[:], r0p_ps[:], AF.Ln)
                lnc14 = rpool.tile([128, SLAB], F32, tag=f"lnc14_{b}")
                nc.vector.tensor_scalar(lnc14[:], lr[:], 14.0, C14L,
                                        OP.mult, OP.add)
                lnc16 = rpool.tile([128, SLAB], F32, tag=f"lnc16_{b}")
                nc.vector.tensor_scalar(lnc16[:], lr[:], 16.0, C16L,
                                        OP.mult, OP.add)
                lnc14_s.append(lnc14)
                lnc16_s.append(lnc16)

                rco_ps = psS.tile([128, SLAB], F32, tag="small")
                nc.tensor.matmul(rco_ps[:], Lrco_s[:, jsl], Rrco_s[:],
                                 start=True, stop=True)
                rco = rpool.tile([128, SLAB], F32, tag=f"rco_{b}")
                nc.scalar.copy(rco[:], rco_ps[:])
                rco_s.append(rco)

            # ---------- phase 1: geometry + CN path ----------
            # log-space formulation: L2 = ln r2; u2 = e^-L2; u = e^-L2/2;
            # 1/(1 + c*r2^-k) = sigmoid(k*L2 - ln c).
            u6_s, u8_s, L2_s = [], [], []
            exp_insts, sig_insts = [], []
            for b in range(JB):
                jsl = slice(b * 128, (b + 1) * 128)
                r2_ps = psA.tile([128, FREE], F32, tag="r2ps")
                for (c0, c1) in _CHUNKS:
                    nc.tensor.matmul(r2_ps[:, c0:c1], Lcat_s[:, jsl],
                                     Rcat_s[:, c0:c1], start=True, stop=True)
                r2c = bpool.tile([128, FREE], F32, tag="chain")
                nc.vector.tensor_scalar_max(r2c[:], r2_ps[:], R2MIN)
                L2 = u2pool.tile([128, FREE], F32, tag=f"L2_{b}")
                nc.scalar.activation(L2[:], r2c[:], AF.Ln)
                L2_s.append(L2)
                u6 = u2pool.tile([128, FREE], F32, tag=f"u6_{b}")
                nc.scalar.activation(u6[:], L2[:], AF.Exp, scale=-3.0)
                u6_s.append(u6)
                u8 = u2pool.tile([128, FREE], F32, tag=f"u8_{b}")
                nc.scalar.activation(u8[:], L2[:], AF.Exp, scale=-4.0)
                u8_s.append(u8)
                # one tile per block, transformed in place:
                # e^{-L2/2} -> *rco -> sigmoid -> *cn-mask
                us = bpool.tile([128, FREE], F32, tag="chain")
                ei = nc.scalar.activation(us[:], L2[:], AF.Exp, scale=-0.5)
                exp_insts.append(ei)
                nc.vector.tensor_tensor(
                    us[:].rearrange("p (i s) -> p i s", s=NS),
                    us[:].rearrange("p (i s) -> p i s", s=NS),
                    _bc_s(rco_s[b][:]), OP.mult)
                sig = nc.scalar.activation(us[:], us[:], AF.Sigmoid,
                                           bias=negk1[:], scale=K1)
                sig_insts.append(sig)
                # The r<=CNTHR mask is dropped: every excluded pair has
                # damp = sigmoid(K1(rco/r-1)) <= ~1.2e-6 (r>40, rco<=6), so
                # including the tails shifts cn by ~1e-3 (~1e-4 on E) and
                # saves one saturated-VectorE pass per block.
                cnred = rpool.tile([128, SLAB], F32, tag=f"cnred_{b}")
                nc.vector.tensor_reduce(
                    cnred[:], us[:].rearrange("p (i s) -> p i s", s=NS),
                    axis=mybir.AxisListType.X, op=OP.add)
                if b == 0:
                    cn_ps = psK.tile([1, SLAB], F32, tag="cnps")
                nc.tensor.matmul(cn_ps[:], ones128x1[:], cnred[:],
                                 start=(b == 0), stop=(b == JB - 1))

            # all sigmoid-set ops must come after every ln/exp-set op
            for sg in sig_insts:
                add_dep_helper(sg.ins, exp_insts[-1].ins, sync=False,
                               reason="batch act tables: sigmoid after exp")
            last_sigmoid = sig

            # ---------- CN collective ----------
            cc_in = dpool.tile([1, SLAB], F32, tag="ccin")
            cc_out = dpool.tile([NCORES, SLAB], F32, tag="ccout")
            cnsl = ppool.tile([1, SLAB], F32, tag="cnsl")
            nc.scalar.copy(cnsl[:], cn_ps[:])
            nc.gpsimd.dma_start(cc_in[:], cnsl[:])
            nc.gpsimd.collective_compute(
                "AllGather", OP.bypass, replica_groups=[list(range(NCORES))],
                ins=[cc_in.opt()], outs=[cc_out.opt()],
            )
            cn_t = ppool.tile([128, JB], F32, tag="cn_t")
            nc.gpsimd.dma_start(
                cn_t[:],
                cc_out[:].rearrange("a b -> (a b)").rearrange("(b p) -> p b", p=128))

            # ---------- phase 1b: A/B chains ----------
            Ared_s, Bred_s = [], []
            for b in range(JB):
                u6 = u6_s[b]
                u8 = u8_s[b]
                L2 = L2_s[b]
                g6 = bpool.tile([128, FREE], F32, tag="chain")
                nc.vector.scalar_tensor_tensor(
                    g6[:].rearrange("p (i s) -> p i s", s=NS),
                    L2[:].rearrange("p (i s) -> p i s", s=NS),
                    float(ALP / 2.0), _bc_s(lnc14_s[b][:]),
                    OP.mult, OP.subtract)
                rec6 = bpool.tile([128, FREE], F32, tag="chain")
                s6i = nc.scalar.activation(rec6[:], g6[:], AF.Sigmoid)
                add_dep_helper(s6i.ins, exp_insts[-1].ins, sync=False,
                               reason="batch act tables")
                term6 = bpool.tile([128, FREE], F32, tag="chain")
                nc.vector.tensor_tensor(term6[:], u6[:], rec6[:], OP.mult)
                Ared = rpool.tile([128, SLAB], F32, tag=f"Ared_{b}")
                nc.vector.tensor_reduce(
                    Ared[:], term6[:].rearrange("p (i s) -> p i s", s=NS),
                    axis=mybir.AxisListType.X, op=OP.add)
                Ared_s.append(Ared)

                g8 = bpool.tile([128, FREE], F32, tag="chain")
                nc.vector.scalar_tensor_tensor(
                    g8[:].rearrange("p (i s) -> p i s", s=NS),
                    L2[:].rearrange("p (i s) -> p i s", s=NS),
                    float((ALP + 2.0) / 2.0), _bc_s(lnc16_s[b][:]),
                    OP.mult, OP.subtract)
                rec8 = bpool.tile([128, FREE], F32, tag="chain")
                s8i = nc.scalar.activation(rec8[:], g8[:], AF.Sigmoid)
                add_dep_helper(s8i.ins, exp_insts[-1].ins, sync=False,
                               reason="batch act tables")
                term8 = bpool.tile([128, FREE], F32, tag="chain")
                nc.vector.tensor_tensor(term8[:], u8[:], rec8[:], OP.mult)
                Bred = rpool.tile([128, SLAB], F32, tag=f"Bred_{b}")
                nc.vector.tensor_reduce(
                    Bred[:], term8[:].rearrange("p (i s) -> p i s", s=NS),
                    axis=mybir.AxisListType.X, op=OP.add)
                Bred_s.append(Bred)

            last_ab_sigmoid = s8i
            # ---------- phase 2: phi for all atoms ----------
            sq_t, ex_t = [], []
            first_exp = None
            for a in range(NG):
                sq = ppool.tile([128, JB], F32, tag=f"sq_{a}")
                si = nc.scalar.activation(sq[:], cn_t[:], AF.Square,
                                          bias=gneg_s[:, a:a + 1])
                add_dep_helper(si.ins, last_ab_sigmoid.ins, sync=False,
                               reason="phi after all A/B sigmoids")
                sq_t.append(sq)
            mn = ppool.tile([128, JB], F32, tag="mn")
            nc.vector.tensor_tensor(mn[:], sq_t[0][:], sq_t[1][:], OP.min)
            for a in range(2, NG):
                nc.vector.tensor_tensor(mn[:], mn[:], sq_t[a][:], OP.min)
            for a in range(NG):
                dt_ = ppool.tile([128, JB], F32, tag=f"dt_{a}")
                nc.vector.tensor_tensor(dt_[:], sq_t[a][:], mn[:], OP.subtract)
                ex = ppool.tile([128, JB], F32, tag=f"ex_{a}")
                ei = nc.scalar.activation(ex[:], dt_[:], AF.Exp, scale=K3)
                if first_exp is None:
                    first_exp = ei
                    add_dep_helper(ei.ins, last_ab_sigmoid.ins, sync=False,
                                   reason="exp after all A/B sigmoids")
                ex_t.append(ex)
            ssum = ppool.tile([128, JB], F32, tag="ssum")
            nc.vector.tensor_tensor(ssum[:], ex_t[0][:], ex_t[1][:], OP.add)
            for a in range(2, NG):
                nc.vector.tensor_tensor(ssum[:], ssum[:], ex_t[a][:], OP.add)
            lss = ppool.tile([128, JB], F32, tag="lss")
            nc.scalar.activation(lss[:], ssum[:], AF.Ln)
            rs = ppool.tile([128, JB], F32, tag="rs")
            nc.scalar.activation(rs[:], lss[:], AF.Exp, scale=-1.0)
            phi_t = []
            for a in range(NG):
                ph = ppool.tile([128, JB], F32, tag=f"phi_{a}")
                nc.vector.tensor_tensor(ph[:], ex_t[a][:], rs[:], OP.mult)
                phi_t.append(ph)

            # ---------- phi^T for the slab ----------
            sqT, exT = [], []
            for a in range(NG):
                s_ = ppool.tile([1, SLAB], F32, tag=f"sqT_{a}")
                si = nc.scalar.activation(s_[:], cnsl[:], AF.Square,
                                          bias=gneg_s[0:1, a:a + 1])
                add_dep_helper(si.ins, last_ab_sigmoid.ins, sync=False,
                               reason="phiT after all A/B sigmoids")
                sqT.append(s_)
            mnT = ppool.tile([1, SLAB], F32, tag="mnT")
            nc.vector.tensor_tensor(mnT[:], sqT[0][:], sqT[1][:], OP.min)
            for a in range(2, NG):
                nc.vector.tensor_tensor(mnT[:], mnT[:], sqT[a][:], OP.min)
            for a in range(NG):
                dT = ppool.tile([1, SLAB], F32, tag=f"dT_{a}")
                nc.vector.tensor_tensor(dT[:], sqT[a][:], mnT[:], OP.subtract)
                e_ = ppool.tile([1, SLAB], F32, tag=f"eT_{a}")
                nc.scalar.activation(e_[:], dT[:], AF.Exp, scale=K3)
                exT.append(e_)
            sT = ppool.tile([1, SLAB], F32, tag="sT")
            nc.vector.tensor_tensor(sT[:], exT[0][:], exT[1][:], OP.add)
            for a in range(2, NG):
                nc.vector.tensor_tensor(sT[:], sT[:], exT[a][:], OP.add)
            lsT = ppool.tile([1, SLAB], F32, tag="lsT")
            nc.scalar.activation(lsT[:], sT[:], AF.Ln)
            rT = ppool.tile([1, SLAB], F32, tag="rT")
            nc.scalar.activation(rT[:], lsT[:], AF.Exp, scale=-1.0)
            phiT_a = []
            for a in range(NG):
                pt = ppool.tile([1, SLAB], F32, tag=f"phiTn_{a}")
                nc.vector.tensor_tensor(pt[:], exT[a][:], rT[:], OP.mult)
                phiT_a.append(pt)

            # ---------- phase 2: Q, G, contractions ----------
            W2_ps = psK.tile([SLAB, NZ * NG], F32, tag="W2")
            for b in range(JB):
                jsl = slice(b * 128, (b + 1) * 128)
                Q = spool.tile([128, NZ * NG], mybir.dt.bfloat16, tag="Q")
                for g in range(NG):
                    nc.vector.tensor_scalar(
                        Q[:, g * NZ:(g + 1) * NZ], ohZ_b[b][:],
                        phi_t[g][:, b:b + 1], None, OP.mult)
                t1 = spool.tile([128, SLAB], F32, tag="g_t1")
                nc.vector.tensor_scalar(t1[:], Bred_s[b][:],
                                        r2r4c_s[:, b:b + 1], None, OP.mult)
                t2 = spool.tile([128, SLAB], F32, tag="g_t2")
                nc.vector.tensor_tensor(t2[:], t1[:], r2r4i_s[:], OP.mult)
                G = spool.tile([128, SLAB], mybir.dt.bfloat16, tag="G")
                nc.vector.scalar_tensor_tensor(
                    G[:], t2[:], CB8, Ared_s[b][:], OP.mult, OP.add)
                nc.tensor.matmul(W2_ps[:], G[:], Q[:],
                                 start=(b == 0), stop=(b == JB - 1))

            PC_ps = psK.tile([SLAB, NZ * NG], F32, tag="PC")
            for a in range(NG):
                phiA_ps = psS.tile([NZ, SLAB], F32, tag="small")
                nc.tensor.matmul(phiA_ps[:], ones1x95[:], phiT_a[a][:],
                                 start=True, stop=True)
                PT = spool.tile([NZ, SLAB], mybir.dt.bfloat16, tag="PT")
                nc.vector.tensor_tensor(PT[:], ohZiT_s[:], phiA_ps[:], OP.mult)
                nc.tensor.matmul(PC_ps[:], PT[:], C2h_a[a][:],
                                 start=(a == 0), stop=False)
                nc.tensor.matmul(PC_ps[:], PT[:], C2l_a[a][:],
                                 start=False, stop=(a == NG - 1))
            PC_s = spool.tile([SLAB, NZ * NG], F32, tag="PCs")
            nc.scalar.copy(PC_s[:], PC_ps[:])

            scr = spool.tile([SLAB, NZ * NG], F32, tag="scr")
            nc.vector.tensor_tensor(scr[:], W2_ps[:], PC_s[:], OP.mult)
            E_col = ppool.tile([SLAB, 1], F32, tag="Ecol")
            nc.vector.tensor_reduce(E_col[:], scr[:],
                                    axis=mybir.AxisListType.X, op=OP.add)
            nc.sync.dma_start(e_part[:], E_col[:, 0])

    _split_excess_waits(nc)
    return nc


# ----------------------------------------------------------------------
# host side
# ----------------------------------------------------------------------

def _check_separable(c6ab):
    t1 = c6ab[..., 1]
    t2 = c6ab[..., 2]
    g = t1[0, 0, :, 0]
    ok = (np.abs(t1 - g[None, None, :, None]).max() == 0.0
          and np.abs(t2 - g[None, None, None, :]).max() == 0.0
          and (c6ab[..., 0] > 0).all())
    return ok, g.astype(np.float32)


def _host_prep(Z, pos, shift_int, cell, c6ab, r0ab, rcov, r2r4):
    f32 = np.float32
    Zi = np.clip(np.asarray(Z).astype(np.int64), 0, NZ - 1)
    pos_b = (np.asarray(pos, f32) / f32(AUTOANG)).astype(f32)
    cell_b = (np.asarray(cell, f32) / f32(AUTOANG)).astype(f32)
    shifts = (np.asarray(shift_int, f32) @ cell_b).astype(f32)
    rcov_z = np.asarray(rcov, f32)[Zi]
    r2r4_z = np.asarray(r2r4, f32)[Zi]

    ok, g = _check_separable(np.asarray(c6ab, f32))
    if not ok:
        return None

    bf16 = ml_dtypes.bfloat16

    def split3(x):
        x = np.asarray(x, np.float64)
        h = x.astype(bf16)
        r = x - h.astype(np.float64)
        m = r.astype(bf16)
        l = (r - m.astype(np.float64)).astype(bf16)
        return h, m, l

    # shared operands.  r2[j,f] = |p_j|^2 - 2 p_j.y_f + |y_f|^2 + self-offset
    # as ONE bf16 matmul with 3-way split operands (24-bit mantissa):
    # rows 0..23  cross term, 8 split-pairs per dim
    # rows 24..26 |p|^2 (L) x ones (R);  27..29 ones (L) x |y|^2 (R)
    # rows 30..93 per-core self-pair offset (sel x D64)
    pj2 = (pos_b.astype(np.float64) ** 2).sum(-1)
    Lcat = np.zeros((94, N), bf16)
    ph, pm, pl = split3(pos_b.T)          # [3 dims, N] each
    p2h, p2m, p2l = split3(pj2)
    onesN = np.ones(N, bf16)
    for d in range(3):
        base = d * 8
        Lcat[base + 0] = ph[d]; Lcat[base + 1] = ph[d]
        Lcat[base + 2] = pm[d]; Lcat[base + 3] = pm[d]
        Lcat[base + 4] = ph[d]; Lcat[base + 5] = pl[d]
        Lcat[base + 6] = pm[d]; Lcat[base + 7] = pl[d]
    Lcat[24] = p2h; Lcat[25] = p2m; Lcat[26] = p2l
    Lcat[27] = onesN; Lcat[28] = onesN; Lcat[29] = onesN
    Lrco = np.stack([rcov_z, np.ones(N, f32)], axis=0)
    oh = np.zeros((N, NZ), f32)
    oh[np.arange(N), Zi] = 1.0
    r0s = np.asarray(r0ab, f32)
    C2 = np.asarray(c6ab, np.float64)[..., 0].transpose(2, 0, 3, 1).reshape(
        NZ * NG, NZ * NG).copy()
    C2h = C2.astype(bf16)
    C2l = (C2 - C2h.astype(np.float64)).astype(bf16)
    gneg = np.broadcast_to(-g[None, :], (128, NG)).copy()
    r2r4c = r2r4_z.reshape(JB, 128).T.copy()

    shared = dict(Lrco=Lrco, r0abT=np.ascontiguousarray(r0s.T),
                  ohZT=np.ascontiguousarray(oh.T), ohZ=oh, r2r4c=r2r4c,
                  C2h=C2h, C2l=C2l, gneg=gneg)

    y_all = pos_b[:, None, :] - shifts[None, :, :]          # [N, S, 3]
    in_maps = []
    for c in range(NCORES):
        isl = slice(c * SLAB, (c + 1) * SLAB)
        y = y_all[isl].reshape(SLAB * NS, 3).astype(f32)    # [(i s), 3]
        y2 = (y.astype(np.float64) ** 2).sum(-1)
        q = -2.0 * y.astype(np.float64)                      # [FREE, 3]
        Rcat = np.zeros((94, FREE), bf16)
        qh, qm, ql = split3(q.T)
        y2h, y2m, y2l = split3(y2)
        onesF = np.ones(FREE, bf16)
        for d in range(3):
            base = d * 8
            Rcat[base + 0] = qh[d]; Rcat[base + 1] = qm[d]
            Rcat[base + 2] = qh[d]; Rcat[base + 3] = qm[d]
            Rcat[base + 4] = ql[d]; Rcat[base + 5] = qh[d]
            Rcat[base + 6] = ql[d]; Rcat[base + 7] = qm[d]
        Rcat[24] = onesF; Rcat[25] = onesF; Rcat[26] = onesF
        Rcat[27] = y2h; Rcat[28] = y2m; Rcat[29] = y2l
        # self-pair offset rows
        Lc = Lcat.copy()
        Lc[30 + np.arange(SLAB), c * SLAB + np.arange(SLAB)] = bf16(SELF_R2)
        Rcat[30 + np.arange(SLAB), np.arange(SLAB) * NS + (NS // 2)] = bf16(1.0)
        Rrco = np.stack([np.ones(SLAB, f32), rcov_z[isl]], axis=0)
        per = dict(Lcat=Lc, Rcat=Rcat, Rrco=Rrco,
                   ohZiT=np.ascontiguousarray(oh[isl].T),
                   r2r4sl=r2r4_z[isl][None, :])
        per.update(shared)
        in_maps.append(per)

    # cheap host check: no real pair beyond CUTOFF (mask was dropped)
    # max |d| <= max|pos_j - y|: bound via norms, exact check is cheap enough
    dmax2 = ((np.abs(pos_b).max(0) + np.abs(y_all).max((0, 1))) ** 2).sum()
    if dmax2 > CUTOFF * CUTOFF:
        # exact check on the full pair tensor
        d = pos_b[None, :, None, :] - pos_b[:, None, None, :] + \
            shifts[None, None, :, :]
        if (d * d).sum(-1).max() > CUTOFF * CUTOFF:
            return None
    return in_maps


def _numpy_fallback(Z, pos, shift_int, cell, c6ab, r0ab, rcov, r2r4):
    """Exact reference math in numpy (f32), used only when the fast-path
    assumptions do not hold."""
    f32 = np.float32
    Zi = np.asarray(Z).astype(np.int64)
    pos_b = np.asarray(pos, f32) / f32(AUTOANG)
    cell_b = np.asarray(cell, f32) / f32(AUTOANG)
    shifts = np.asarray(shift_int, f32) @ cell_b
    d = pos_b[None, :, None, :] - pos_b[:, None, None, :] + shifts[None, None, :, :]
    r2 = (d * d).sum(-1)
    mask = r2 > 1e-8
    r = np.sqrt(np.where(mask, r2, 1.0))
    in_cut = mask & (r <= CUTOFF)
    rcov_z = np.asarray(rcov, f32)[Zi]
    rco = rcov_z[:, None] + rcov_z[None, :]
    dmp = 1.0 / (1.0 + np.exp(-K1 * (rco[:, :, None] / r - 1.0)))
    cn = np.where(mask & (r <= CNTHR), dmp, 0.0).sum(axis=(1, 2))
    tbl = np.asarray(c6ab, f32)[Zi[:, None], Zi[None, :]]
    c6r = tbl[..., 0]
    valid = c6r > 0.0
    dcn = (cn[:, None, None, None] - tbl[..., 1]) ** 2 + \
          (cn[None, :, None, None] - tbl[..., 2]) ** 2
    dmin = np.where(valid, dcn, 1e10).min(axis=(-2, -1), keepdims=True)
    w = np.where(valid, np.exp(K3 * (dcn - dmin)), 0.0)
    c6 = (c6r * w).sum((-2, -1)) / np.maximum(w.sum((-2, -1)), 1e-20)
    r2r4_z = np.asarray(r2r4, f32)[Zi]
    c8 = 3.0 * c6 * r2r4_z[:, None] * r2r4_z[None, :]
    r0 = np.asarray(r0ab, f32)[Zi[:, None], Zi[None, :]]
    r6 = np.where(mask, r2, 1.0) ** 3
    r8 = r6 * np.where(mask, r2, 1.0)
    t6 = (r / (RS6 * r0[:, :, None])) ** (-ALP)
    t8 = (r / (RS18 * r0[:, :, None])) ** (-(ALP + 2.0))
    e6 = S6 * c6[:, :, None] / r6 / (1.0 + 6.0 * t6)
    e8 = S18 * c8[:, :, None] / r8 / (1.0 + 6.0 * t8)
    E = -0.5 * np.where(in_cut, e6 + e8, 0.0).sum(dtype=np.float64)
    return np.asarray(np.float32(AUTOEV * E))


_PROGRAM_CACHE = {}


def kernel(**inputs) -> np.ndarray:
    inputs = {k: np.asarray(v) for k, v in inputs.items()}
    shapes_ok = (inputs["pos"].shape == (N, 3)
                 and inputs["shift_int"].shape == (NS, 3)
                 and inputs["c6ab"].shape == (NZ, NZ, NG, NG, 3))
    in_maps = _host_prep(**inputs) if shapes_ok else None
    if in_maps is None:
        return _numpy_fallback(**inputs)

    if "nc" not in _PROGRAM_CACHE:
        _PROGRAM_CACHE["nc"] = build_program()
    nc = _PROGRAM_CACHE["nc"]

    trace = bool(os.environ.get("D3_TRACE"))
    res = run_bass_kernel_spmd(nc, in_maps, list(range(NCORES)), trace=trace)
    _PROGRAM_CACHE["last_exec_time_ns"] = res.exec_time_ns
    _PROGRAM_CACHE["last_results"] = res
    e = np.zeros((), np.float64)
    for c in range(NCORES):
        e += res.results[c]["e_part"].astype(np.float64).sum()
    out = np.float32(-0.5 * AUTOEV * S6 * e)
    return np.asarray(out)


if __name__ == "__main__":
    # quick self-run against random inputs is not possible standalone;
    # use test.py next to reference.py
    nc = build_program()
    print("program built:",
          sum(len(b.instructions) for f in nc.m.functions for b in f.blocks),
          "instructions")



# revision 10
# speedup vs baseline: 1.0595x; 1.0595x over previous
"""DFT-D3 (zero damping, static all-pairs) two-body dispersion energy on 8
Trainium2 NeuronCores — v2 (optimized schedule).

Structure (i-slab sharding, 64 atoms per core):
  - r2[j,(i,s)] for the slab via one bf16 TensorE matmul chain per 128-j
    block (3-way bf16-split operands ~24-bit; self-pair offset rows merged
    into the matmul; tiny eps folded into |y|^2 so no clamp pass is needed).
  - Log-space rationals: L2 = ln r2 (ScalarE reads PSUM directly);
    damping factors are sigmoids of affine functions of L2.
  - CN path: damp = sigmoid(K1*(rco*e^{-L2/2} - 1)); the sigmoid's
    accum_out produces the full-tile per-partition sum = this core's
    contribution to cn[j] for the 128 j's of the block (valid because the
    shift set is symmetric), giving a [128,4] partial vector -> one
    AllReduce(add). No reduce/transpose matmuls needed.
  - Slab extraction from the reduced cn (per-core data, same program):
    cn_col[i] = reduce_b(B4 * (S128^T @ cn_t)) with one-hot S128/B4 inputs.
  - A/B chains: g6/g8 computed on GpSimd (scalar_tensor_tensor), u6/u8 and
    sigmoids on ScalarE (batched by activation-table set: log/sig/log/sig),
    products in bf16 (2x DVE mode), merged [128, 2*1728] tiles.
  - Phase 2 (c6 interpolation) exploits the separable c6ab grid structure;
    phi chains fused into [128,20] / [64,5] tiles; E reduced to a scalar
    on-device (one-descriptor output DMA).
"""
import os
os.environ.setdefault("JAX_PLATFORMS", "cpu")

import math
import numpy as np
import ml_dtypes

import concourse.bass as bass
import concourse.mybir as mybir
from concourse.tile import TileContext, add_dep_helper
from concourse.bass_utils import run_bass_kernel_spmd

F32 = mybir.dt.float32
BF16 = mybir.dt.bfloat16
AF = mybir.ActivationFunctionType
OP = mybir.AluOpType

# D3 constants
AUTOANG = 0.52917726
AUTOEV = 27.21138505
K1, K3 = 16.0, -4.0
CUTOFF, CNTHR = 95.0, 40.0
S6, RS6, S18, RS18, ALP = 1.0, 1.217, 0.722, 1.0, 14.0

N = 512
NS = 27
NCORES = 8
SLAB = N // NCORES           # 64
JB = 4                       # j blocks of 128
FREE = SLAB * NS             # 1728
HALF = FREE // 2             # 864
NZ = 95
NG = 5

SELF_R2 = 1.0e4
R2EPS = 1.0e-4
C14L = float(math.log(6.0) + 14.0 * math.log(RS6))
C16L = float(math.log(6.0) + 16.0 * math.log(RS18))
CB8 = float(3.0 * S18)

# f32 pack PK1 [128, C1] column offsets
_OHZT = 0                      # [95, 512]
_LRCO = 512                    # [2, 512]
_R0ABT = 1024                  # [95, 95]
_OHZIT = 1119                  # [95, 64]
_RRCO = 1183                   # [2, 64]
_R2R4I = 1247                  # [128, 64]
_R2R4C = 1311                  # [128, 4]
_G20 = 1315                    # [128, 20]
_OHZ = 1335                    # [128, 380] (4 x 95)
_S128 = 1715                   # [128, 64]
_B4 = 1779                     # [64, 4]
_I64 = 1783                    # [64, 64]
_G5 = 1847                     # [64, 5]
C1 = 1852

# bf16 pack PKB [95, CB] column offsets
_LCAT = 0                      # [94, 512]
_RCAT = 512                    # [94, 1728]
_C2H = 2240                    # [95, 5*475]
_C2L = 4615                    # [95, 5*475]
CB = 6990


def _bc_s(ap2d, s=NS):
    """[128, M] AP -> [128, M, s] with stride-0 broadcast over s."""
    a3 = ap2d[:, :, None]
    new = [list(a3.ap[0]), list(a3.ap[1]), [0, s]]
    return bass.AP(a3.tensor, a3.offset, new)


def _bc_mid(ap2d, k):
    """[P, M] AP -> 3D [P, k(bcast stride-0), M]."""
    return bass.AP(ap2d.tensor, ap2d.offset,
                   [list(ap2d.ap[0]), [0, k], list(ap2d.ap[1])])


def _split_excess_waits(nc, max_waits=1):
    """This walrus build accepts at most one sync wait per instruction;
    Tile's tail drain can carry several. Hoist excess waits onto inserted
    drains on the same engine (sequential waits == conjunction)."""
    n_split = 0
    for f in nc.m.functions:
        for b in f.blocks:
            new_list = []
            changed = False
            for ins in b.instructions:
                si = ins.sync_info
                if si is not None:
                    waits = list(si.on_wait or [])
                    updates = list(si.on_update or [])
                    if len(waits) > max_waits:
                        excess = waits[: len(waits) - max_waits]
                        keep = waits[len(waits) - max_waits:]
                        for w in excess:
                            d = mybir.InstDrain(
                                name=f"I-waitsplit-{n_split}", ins=[], outs=[])
                            n_split += 1
                            d.engine = ins.engine
                            d.sync_info = mybir.SyncInfo(on_wait=[w], on_update=[])
                            new_list.append(d)
                            changed = True
                        ins.sync_info = mybir.SyncInfo(
                            on_wait=list(keep), on_update=list(updates))
                new_list.append(ins)
            if changed:
                b.instructions = new_list
    return n_split


_orig_clear_sems = bass.Bass.clear_and_free_semaphores


def _chunked_clear_sems(self, sems, _chunk=4):
    nums = sorted(s.num if hasattr(s, "num") else s for s in sems)
    for i in range(0, len(nums), _chunk):
        _orig_clear_sems(self, nums[i:i + _chunk])


bass.Bass.clear_and_free_semaphores = _chunked_clear_sems


def build_program():
    nc = bass.Bass(num_devices=NCORES)

    PKB = nc.dram_tensor("PKB", [NZ, CB], BF16, kind="ExternalInput")
    PK1 = nc.dram_tensor("PK1", [128, C1], F32, kind="ExternalInput")
    e_out = nc.dram_tensor("e_out", [1], F32, kind="ExternalOutput")

    with TileContext(nc) as tc:
        with (
            tc.tile_pool(name="const", bufs=1) as cpool,
            tc.tile_pool(name="L2p", bufs=1) as l2pool,
            tc.tile_pool(name="usp", bufs=4) as uspool,
            tc.tile_pool(name="up", bufs=1) as upool,
            tc.tile_pool(name="gp", bufs=3) as gpool,
            tc.tile_pool(name="recp", bufs=2) as recpool,
            tc.tile_pool(name="mp", bufs=2) as mpool,
            tc.tile_pool(name="sm", bufs=3) as spool,
            tc.tile_pool(name="ph2", bufs=1) as ppool,
            tc.tile_pool(name="psA", bufs=2, space="PSUM") as psA,
            tc.tile_pool(name="psS", bufs=2, space="PSUM") as psS,
            tc.tile_pool(name="psW", bufs=1, space="PSUM") as psW,
            tc.tile_pool(name="psP", bufs=1, space="PSUM") as psP,
            tc.tile_pool(name="dram", bufs=1, space="DRAM") as dpool,
        ):
            # ---------- input DMAs: 2 packed tensors, split across queues
            pkb_s = cpool.tile([NZ, CB], BF16, tag="pkb")
            # critical first: Lcat+Rcat columns (phase-1 matmul operands)
            nc.sync.dma_start(pkb_s[:, 0:_C2H], PKB[:, 0:_C2H])
            pk1_s = cpool.tile([128, C1], F32, tag="pk1")
            nc.gpsimd.dma_start(pk1_s[:], PK1[:])
            # phase-2-only tables later, on a different queue
            nc.scalar.dma_start(pkb_s[:, _C2H:], PKB[:, _C2H:])

            # slices of the packs
            Lcat = pkb_s[0:94, _LCAT:_LCAT + N]
            Rcat = pkb_s[0:94, _RCAT:_RCAT + FREE]
            ohZT = pk1_s[0:NZ, _OHZT:_OHZT + N]
            Lrco = pk1_s[0:2, _LRCO:_LRCO + N]
            r0abT = pk1_s[0:NZ, _R0ABT:_R0ABT + NZ]
            ohZiT = pk1_s[0:NZ, _OHZIT:_OHZIT + SLAB]
            Rrco = pk1_s[0:2, _RRCO:_RRCO + SLAB]
            r2r4i = pk1_s[:, _R2R4I:_R2R4I + SLAB]
            r2r4c = pk1_s[:, _R2R4C:_R2R4C + JB]
            g20 = pk1_s[:, _G20:_G20 + JB * NG]
            S128 = pk1_s[:, _S128:_S128 + SLAB]
            B4 = pk1_s[0:SLAB, _B4:_B4 + JB]
            I64 = pk1_s[0:SLAB, _I64:_I64 + SLAB]
            G5 = pk1_s[0:SLAB, _G5:_G5 + NG]

            ones1x95 = cpool.tile([1, NZ], F32, tag="ones95")
            nc.gpsimd.memset(ones1x95[:], 1.0)
            ones64 = cpool.tile([SLAB, 1], F32, tag="ones64")
            nc.gpsimd.memset(ones64[:], 1.0)
            negk1 = cpool.tile([128, 1], F32, tag="negk1")
            nc.gpsimd.memset(negk1[:], -K1)

            # ---------- prep matmuls ----------
            # R1[z1, i] = r0ab[z1, Z_i]
            R1_ps = psS.tile([NZ, SLAB], F32, tag="small")
            nc.tensor.matmul(R1_ps[:], r0abT, ohZiT, start=True, stop=True)
            R1_s = cpool.tile([NZ, SLAB], F32, tag="R1")
            nc.vector.tensor_copy(R1_s[:], R1_ps[:])

            log_batch1 = []  # all natural_log_exp-set ops of round 1
            lnc14_s, lnc16_s, rco_s = [], [], []
            for b in range(JB):
                jsl = slice(b * 128, (b + 1) * 128)
                r0p_ps = psS.tile([128, SLAB], F32, tag="small")
                nc.tensor.matmul(r0p_ps[:], ohZT[:, jsl], R1_s[:],
                                 start=True, stop=True)
                lr = spool.tile([128, SLAB], F32, tag="lnr0")
                li = nc.scalar.activation(lr[:], r0p_ps[:], AF.Ln)
                log_batch1.append(li)
                lnc14 = cpool.tile([128, SLAB], F32, tag=f"lnc14_{b}")
                nc.vector.tensor_scalar(lnc14[:], lr[:], 2.0, C14L / 7.0,
                                        OP.mult, OP.add)
                lnc16 = cpool.tile([128, SLAB], F32, tag=f"lnc16_{b}")
                nc.vector.tensor_scalar(lnc16[:], lr[:], 2.0, C16L / 8.0,
                                        OP.mult, OP.add)
                lnc14_s.append(lnc14)
                lnc16_s.append(lnc16)

                rco_ps = psS.tile([128, SLAB], F32, tag="small")
                nc.tensor.matmul(rco_ps[:], Lrco[:, jsl], Rrco,
                                 start=True, stop=True)
                rco = cpool.tile([128, SLAB], F32, tag=f"rco_{b}")
                nc.vector.tensor_copy(rco[:], rco_ps[:])
                rco_s.append(rco)

            # ---------- phase 1a: r2, L2, CN partials ----------
            cnpart = ppool.tile([128, JB], F32, tag="cnpart")
            L2_s, g_s = [], []
            sigcn_insts = []
            for b in range(JB):
                jsl = slice(b * 128, (b + 1) * 128)
                L2 = l2pool.tile([128, FREE], F32, tag=f"L2_{b}")
                L2_s.append(L2)
                for h in range(2):
                    r2_ps = psA.tile([128, HALF], F32, tag="r2ps")
                    c0 = h * HALF
                    nc.tensor.matmul(r2_ps[:, 0:512], Lcat[:, jsl],
                                     Rcat[:, c0:c0 + 512],
                                     start=True, stop=True)
                    nc.tensor.matmul(r2_ps[:, 512:HALF], Lcat[:, jsl],
                                     Rcat[:, c0 + 512:c0 + HALF],
                                     start=True, stop=True)
                    li = nc.scalar.activation(L2[:, c0:c0 + HALF], r2_ps[:],
                                              AF.Ln)
                    log_batch1.append(li)
                # g6/g8 on GpSimd as soon as L2(b) exists (no act table)
                g = gpool.tile([128, 2 * FREE], F32, tag="g68")
                g_s.append(g)
                nc.gpsimd.tensor_tensor(
                    g[:, 0:FREE].rearrange("p (i s) -> p i s", s=NS),
                    L2[:].rearrange("p (i s) -> p i s", s=NS),
                    _bc_s(lnc14_s[b][:]), OP.subtract)
                nc.gpsimd.tensor_tensor(
                    g[:, FREE:2 * FREE].rearrange("p (i s) -> p i s", s=NS),
                    L2[:].rearrange("p (i s) -> p i s", s=NS),
                    _bc_s(lnc16_s[b][:]), OP.subtract)
                # cn chain: us = e^{-L2/2}; usr = us*rco; sigmoid+accum
                us = uspool.tile([128, FREE], F32, tag="us")
                ei = nc.scalar.activation(us[:], L2[:], AF.Exp, scale=-0.5)
                log_batch1.append(ei)
                nc.vector.tensor_tensor(
                    us[:].rearrange("p (i s) -> p i s", s=NS),
                    us[:].rearrange("p (i s) -> p i s", s=NS),
                    _bc_s(rco_s[b][:]), OP.mult)
                si = nc.scalar.activation(us[:], us[:], AF.Sigmoid,
                                          bias=negk1[:], scale=K1,
                                          accum_out=cnpart[:, b:b + 1])
                sigcn_insts.append(si)

            for si in sigcn_insts:
                add_dep_helper(si.ins, log_batch1[-1].ins, sync=False,
                               reason="sigmoid set after log set (round 1)")
                add_dep_helper(si.ins, log_batch1[3].ins, sync=False,
                               reason="after prep lnr0 Ln ops too")

            # ---------- CN collective (AllReduce of [128,4] partials) ----
            cc_in = dpool.tile([128, JB], F32, tag="ccin")
            cc_out = dpool.tile([128, JB], F32, tag="ccout")
            nc.gpsimd.dma_start(cc_in[:], cnpart[:])
            nc.gpsimd.collective_compute(
                "AllReduce", OP.add, replica_groups=[list(range(NCORES))],
                ins=[cc_in.opt()], outs=[cc_out.opt()],
            )
            cn_t = ppool.tile([128, JB], F32, tag="cn_t")
            nc.gpsimd.dma_start(cn_t[:], cc_out[:])

            # ---------- phase 1b: u6/u8 exps, sigmoids, products, reduces
            e_batch2 = []
            u_s = []
            for b in range(JB):
                u = upool.tile([128, 2 * FREE], BF16, tag=f"u68_{b}")
                u_s.append(u)
                e3 = nc.scalar.activation(u[:, 0:FREE], L2_s[b][:],
                                          AF.Exp, scale=-3.0)
                e4 = nc.scalar.activation(u[:, FREE:2 * FREE], L2_s[b][:],
                                          AF.Exp, scale=-4.0)
                add_dep_helper(e3.ins, sigcn_insts[-1].ins, sync=False,
                               reason="log set round 2 after cn sigmoids")
                add_dep_helper(e4.ins, sigcn_insts[-1].ins, sync=False,
                               reason="log set round 2 after cn sigmoids")
                e_batch2 += [e3, e4]

            AB_s = []
            last_sig68 = None
            for b in range(JB):
                rec = recpool.tile([128, 2 * FREE], BF16, tag="rec")
                s6i = nc.scalar.activation(rec[:, 0:FREE], g_s[b][:, 0:FREE],
                                           AF.Sigmoid, scale=float(ALP / 2.0))
                s8i = nc.scalar.activation(rec[:, FREE:2 * FREE],
                                           g_s[b][:, FREE:2 * FREE],
                                           AF.Sigmoid,
                                           scale=float((ALP + 2.0) / 2.0))
                add_dep_helper(s6i.ins, e_batch2[-1].ins, sync=False,
                               reason="sigmoid set round 2 after exps")
                add_dep_helper(s8i.ins, e_batch2[-1].ins, sync=False,
                               reason="sigmoid set round 2 after exps")
                last_sig68 = s8i
                m = mpool.tile([128, 2 * FREE], BF16, tag="m68")
                nc.vector.tensor_tensor(m[:], u_s[b][:], rec[:], OP.mult)
                AB = ppool.tile([128, 2 * SLAB], F32, tag=f"AB_{b}")
                nc.vector.tensor_reduce(
                    AB[:], m[:].rearrange("p (k s) -> p k s", s=NS),
                    axis=mybir.AxisListType.X, op=OP.add)
                AB_s.append(AB)

            # ---------- phase 2: phi for all atoms ([128, (a,b)=20]) ----
            d20 = ppool.tile([128, JB * NG], F32, tag="d20")
            nc.vector.tensor_tensor(
                d20[:].rearrange("p (a b) -> p a b", a=NG),
                _bc_mid(cn_t[:], NG), g20.rearrange("p (a b) -> p a b", a=NG),
                OP.subtract)
            sq20 = ppool.tile([128, JB * NG], F32, tag="sq20")
            nc.vector.tensor_tensor(sq20[:], d20[:], d20[:], OP.mult)
            mn4 = ppool.tile([128, JB], F32, tag="mn4")
            nc.vector.tensor_reduce(
                mn4[:], sq20[:].rearrange("p (a b) -> p b a", a=NG),
                axis=mybir.AxisListType.X, op=OP.min)
            dt20 = ppool.tile([128, JB * NG], F32, tag="dt20")
            nc.vector.tensor_tensor(
                dt20[:].rearrange("p (a b) -> p a b", a=NG),
                sq20[:].rearrange("p (a b) -> p a b", a=NG),
                _bc_mid(mn4[:], NG), OP.subtract)
            ex20 = ppool.tile([128, JB * NG], F32, tag="ex20")
            xi = nc.scalar.activation(ex20[:], dt20[:], AF.Exp, scale=K3)
            add_dep_helper(xi.ins, last_sig68.ins, sync=False,
                           reason="log set round 3 after A/B sigmoids")
            ss4 = ppool.tile([128, JB], F32, tag="ss4")
            nc.vector.tensor_reduce(
                ss4[:], ex20[:].rearrange("p (a b) -> p b a", a=NG),
                axis=mybir.AxisListType.X, op=OP.add)
            rs4 = ppool.tile([128, JB], F32, tag="rs4")
            nc.vector.reciprocal(rs4[:], ss4[:])
            phi20 = ppool.tile([128, JB * NG], F32, tag="phi20")
            nc.vector.tensor_tensor(
                phi20[:].rearrange("p (a b) -> p a b", a=NG),
                ex20[:].rearrange("p (a b) -> p a b", a=NG),
                _bc_mid(rs4[:], NG), OP.mult)

            # ---------- slab cn + phi^T ([64, 5]-land) ----------
            csel_ps = psS.tile([SLAB, JB], F32, tag="small")
            nc.tensor.matmul(csel_ps[:], S128, cn_t[:], start=True, stop=True)
            csel = ppool.tile([SLAB, JB], F32, tag="csel")
            nc.vector.tensor_tensor(csel[:], csel_ps[:], B4, OP.mult)
            cn_col = ppool.tile([SLAB, 1], F32, tag="cncol")
            nc.vector.tensor_reduce(cn_col[:], csel[:],
                                    axis=mybir.AxisListType.X, op=OP.add)
            dP = ppool.tile([SLAB, NG], F32, tag="dP")
            cnb = cn_col[:, :]
            cnb3 = bass.AP(cnb.tensor, cnb.offset,
                           [list(cnb.ap[0]), [0, NG]])
            nc.vector.tensor_tensor(dP[:], cnb3, G5, OP.subtract)
            sqP = ppool.tile([SLAB, NG], F32, tag="sqP")
            nc.vector.tensor_tensor(sqP[:], dP[:], dP[:], OP.mult)
            mnP = ppool.tile([SLAB, 1], F32, tag="mnP")
            nc.vector.tensor_reduce(mnP[:], sqP[:],
                                    axis=mybir.AxisListType.X, op=OP.min)
            mnP3 = bass.AP(mnP[:, :].tensor, mnP[:, :].offset,
                           [list(mnP[:, :].ap[0]), [0, NG]])
            dtP = ppool.tile([SLAB, NG], F32, tag="dtP")
            nc.vector.tensor_tensor(dtP[:], sqP[:], mnP3, OP.subtract)
            exP = ppool.tile([SLAB, NG], F32, tag="exP")
            xi2 = nc.scalar.activation(exP[:], dtP[:], AF.Exp, scale=K3)
            add_dep_helper(xi2.ins, last_sig68.ins, sync=False,
                           reason="log set round 3 after A/B sigmoids")
            sP = ppool.tile([SLAB, 1], F32, tag="sP")
            nc.vector.tensor_reduce(sP[:], exP[:],
                                    axis=mybir.AxisListType.X, op=OP.add)
            rP = ppool.tile([SLAB, 1], F32, tag="rP")
            nc.vector.reciprocal(rP[:], sP[:])
            rP3 = bass.AP(rP[:, :].tensor, rP[:, :].offset,
                          [list(rP[:, :].ap[0]), [0, NG]])
            phiP = ppool.tile([SLAB, NG], F32, tag="phiP")
            nc.vector.tensor_tensor(phiP[:], exP[:], rP3, OP.mult)
            # transpose phiP -> [5, 64] via matmul with identity
            phiT_ps = psS.tile([NG, SLAB], F32, tag="small")
            nc.tensor.matmul(phiT_ps[:], phiP[:], I64, start=True, stop=True)
            phiT5 = ppool.tile([NG, SLAB], F32, tag="phiT5")
            nc.vector.tensor_copy(phiT5[:], phiT_ps[:])
            phiT_a = []
            for a in range(NG):
                pt = ppool.tile([1, SLAB], F32, tag=f"phiT_{a}")
                nc.gpsimd.dma_start(pt[:], phiT5[a:a + 1, :])
                phiT_a.append(pt)

            # ---------- phase 2: Q, G, contractions ----------
            W2_ps = psW.tile([SLAB, NZ * NG], F32, tag="W2")
            for b in range(JB):
                Q = spool.tile([128, NZ * NG], BF16, tag="Q")
                for a in range(NG):
                    nc.vector.tensor_scalar(
                        Q[:, a * NZ:(a + 1) * NZ],
                        pk1_s[:, _OHZ + b * NZ:_OHZ + (b + 1) * NZ],
                        phi20[:, a * JB + b:a * JB + b + 1], None, OP.mult)
                t1 = spool.tile([128, SLAB], F32, tag="g_t1")
                nc.vector.tensor_scalar(t1[:], AB_s[b][:, SLAB:2 * SLAB],
                                        r2r4c[:, b:b + 1], None, OP.mult)
                t2 = spool.tile([128, SLAB], F32, tag="g_t2")
                nc.vector.tensor_tensor(t2[:], t1[:], r2r4i, OP.mult)
                G = spool.tile([128, SLAB], BF16, tag="G")
                nc.vector.scalar_tensor_tensor(
                    G[:], t2[:], CB8, AB_s[b][:, 0:SLAB], OP.mult, OP.add)
                nc.tensor.matmul(W2_ps[:], G[:], Q[:],
                                 start=(b == 0), stop=(b == JB - 1))

            PC_ps = psP.tile([SLAB, NZ * NG], F32, tag="PC")
            for a in range(NG):
                phiA_ps = psS.tile([NZ, SLAB], F32, tag="small")
                nc.tensor.matmul(phiA_ps[:], ones1x95[:],
                                 phiT_a[a][:], start=True, stop=True)
                PT = spool.tile([NZ, SLAB], BF16, tag="PT")
                nc.vector.tensor_tensor(PT[:], ohZiT, phiA_ps[:], OP.mult)
                nc.tensor.matmul(PC_ps[:], PT[:],
                                 pkb_s[:, _C2H + a * NZ * NG:
                                       _C2H + (a + 1) * NZ * NG],
                                 start=(a == 0), stop=False)
                nc.tensor.matmul(PC_ps[:], PT[:],
                                 pkb_s[:, _C2L + a * NZ * NG:
                                       _C2L + (a + 1) * NZ * NG],
                                 start=False, stop=(a == NG - 1))
            PC_s = spool.tile([SLAB, NZ * NG], F32, tag="PCs")
            nc.vector.tensor_copy(PC_s[:], PC_ps[:])

            scr = spool.tile([SLAB, NZ * NG], F32, tag="scr")
            nc.vector.tensor_tensor(scr[:], W2_ps[:], PC_s[:], OP.mult)
            E_col = ppool.tile([SLAB, 1], F32, tag="Ecol")
            nc.vector.tensor_reduce(E_col[:], scr[:],
                                    axis=mybir.AxisListType.X, op=OP.add)
            E_ps = psS.tile([1, 1], F32, tag="small")
            nc.tensor.matmul(E_ps[:], ones64[:], E_col[:],
                             start=True, stop=True)
            E_s = ppool.tile([1, 1], F32, tag="Es")
            nc.scalar.copy(E_s[:], E_ps[:])
            nc.sync.dma_start(e_out[:], E_s[0, :])

    _split_excess_waits(nc)
    return nc


# ----------------------------------------------------------------------
# host side
# ----------------------------------------------------------------------

def _check_separable(c6ab):
    t1 = c6ab[..., 1]
    t2 = c6ab[..., 2]
    g = t1[0, 0, :, 0]
    ok = (np.abs(t1 - g[None, None, :, None]).max() == 0.0
          and np.abs(t2 - g[None, None, None, :]).max() == 0.0
          and (c6ab[..., 0] > 0).all())
    return ok, g.astype(np.float32)


def _host_prep(Z, pos, shift_int, cell, c6ab, r0ab, rcov, r2r4):
    f32 = np.float32
    Zi = np.clip(np.asarray(Z).astype(np.int64), 0, NZ - 1)
    pos_b = (np.asarray(pos, f32) / f32(AUTOANG)).astype(f32)
    cell_b = (np.asarray(cell, f32) / f32(AUTOANG)).astype(f32)
    shifts = (np.asarray(shift_int, f32) @ cell_b).astype(f32)
    rcov_z = np.asarray(rcov, f32)[Zi]
    r2r4_z = np.asarray(r2r4, f32)[Zi]

    # the shift-set symmetry (s -> -s closure) is required for the
    # AllReduce cn formulation; verify cheaply
    sh = np.asarray(shift_int, np.int64)
    sset = set(map(tuple, sh.tolist()))
    if not all(tuple(-np.array(t)) in sset for t in sset):
        return None

    ok, g = _check_separable(np.asarray(c6ab, f32))
    if not ok:
        return None

    bf16 = ml_dtypes.bfloat16

    def split3(x):
        x = np.asarray(x, np.float64)
        h = x.astype(bf16)
        r = x - h.astype(np.float64)
        m = r.astype(bf16)
        l = (r - m.astype(np.float64)).astype(bf16)
        return h, m, l

    # ---- bf16 pack: Lcat / Rcat / C2 tables ----
    pj2 = (pos_b.astype(np.float64) ** 2).sum(-1)
    Lcat = np.zeros((94, N), bf16)
    ph, pm, pl = split3(pos_b.T)
    p2h, p2m, p2l = split3(pj2)
    onesN = np.ones(N, bf16)
    for d in range(3):
        base = d * 8
        Lcat[base + 0] = ph[d]; Lcat[base + 1] = ph[d]
        Lcat[base + 2] = pm[d]; Lcat[base + 3] = pm[d]
        Lcat[base + 4] = ph[d]; Lcat[base + 5] = pl[d]
        Lcat[base + 6] = pm[d]; Lcat[base + 7] = pl[d]
    Lcat[24] = p2h; Lcat[25] = p2m; Lcat[26] = p2l
    Lcat[27] = onesN; Lcat[28] = onesN; Lcat[29] = onesN

    C2 = np.asarray(c6ab, np.float64)[..., 0].transpose(2, 0, 3, 1).reshape(
        NZ * NG, NZ * NG)
    # reorder rows (a, z) -> [z, (a, z2g)] layout: row z holds a-blocks
    C2h_f = C2.astype(bf16)
    C2l_f = (C2 - C2h_f.astype(np.float64)).astype(bf16)
    # [NZ*NG, NZ*NG] with row index (a*NZ + z) -> [NZ, NG*NZ*NG]
    C2h_r = np.ascontiguousarray(
        C2h_f.reshape(NG, NZ, NZ * NG).transpose(1, 0, 2).reshape(NZ, NG * NZ * NG))
    C2l_r = np.ascontiguousarray(
        C2l_f.reshape(NG, NZ, NZ * NG).transpose(1, 0, 2).reshape(NZ, NG * NZ * NG))

    oh = np.zeros((N, NZ), f32)
    oh[np.arange(N), Zi] = 1.0
    r0s = np.asarray(r0ab, f32)

    # ---- shared part of f32 pack ----
    PK1_shared = np.zeros((128, C1), f32)
    PK1_shared[0:NZ, _OHZT:_OHZT + N] = oh.T
    PK1_shared[0:2, _LRCO:_LRCO + N] = np.stack(
        [rcov_z, np.ones(N, f32)], axis=0)
    PK1_shared[0:NZ, _R0ABT:_R0ABT + NZ] = r0s.T
    PK1_shared[:, _R2R4C:_R2R4C + JB] = r2r4_z.reshape(JB, 128).T
    PK1_shared[:, _G20:_G20 + JB * NG] = np.repeat(g, JB)[None, :]
    PK1_shared[:, _OHZ:_OHZ + JB * NZ] = np.concatenate(
        [oh[b * 128:(b + 1) * 128, :] for b in range(JB)], axis=1)
    PK1_shared[0:SLAB, _I64:_I64 + SLAB] = np.eye(SLAB, dtype=f32)
    PK1_shared[0:SLAB, _G5:_G5 + NG] = np.broadcast_to(g[None, :], (SLAB, NG))

    y_all = pos_b[:, None, :] - shifts[None, :, :]          # [N, S, 3]
    in_maps = []
    for c in range(NCORES):
        isl = slice(c * SLAB, (c + 1) * SLAB)
        y = y_all[isl].reshape(FREE, 3).astype(f32)
        y2 = (y.astype(np.float64) ** 2).sum(-1) + R2EPS
        q = -2.0 * y.astype(np.float64)
        Rcat = np.zeros((94, FREE), bf16)
        qh, qm, ql = split3(q.T)
        y2h, y2m, y2l = split3(y2)
        onesF = np.ones(FREE, bf16)
        for d in range(3):
            base = d * 8
            Rcat[base + 0] = qh[d]; Rcat[base + 1] = qm[d]
            Rcat[base + 2] = qh[d]; Rcat[base + 3] = qm[d]
            Rcat[base + 4] = ql[d]; Rcat[base + 5] = qh[d]
            Rcat[base + 6] = ql[d]; Rcat[base + 7] = qm[d]
        Rcat[24] = onesF; Rcat[25] = onesF; Rcat[26] = onesF
        Rcat[27] = y2h; Rcat[28] = y2m; Rcat[29] = y2l
        Lc = Lcat.copy()
        Lc[30 + np.arange(SLAB), c * SLAB + np.arange(SLAB)] = bf16(SELF_R2)
        Rcat[30 + np.arange(SLAB), np.arange(SLAB) * NS + (NS // 2)] = bf16(1.0)

        PKBc = np.zeros((NZ, CB), bf16)
        PKBc[0:94, _LCAT:_LCAT + N] = Lc
        PKBc[0:94, _RCAT:_RCAT + FREE] = Rcat
        PKBc[:, _C2H:_C2H + NG * NZ * NG] = C2h_r
        PKBc[:, _C2L:_C2L + NG * NZ * NG] = C2l_r

        PK1c = PK1_shared.copy()
        PK1c[0:NZ, _OHZIT:_OHZIT + SLAB] = oh[isl].T
        PK1c[0:2, _RRCO:_RRCO + SLAB] = np.stack(
            [np.ones(SLAB, f32), rcov_z[isl]], axis=0)
        PK1c[:, _R2R4I:_R2R4I + SLAB] = np.broadcast_to(
            r2r4_z[isl][None, :], (128, SLAB))
        p0 = 64 * (c % 2)
        bsel = c // 2
        S128m = np.zeros((128, SLAB), f32)
        S128m[p0 + np.arange(SLAB), np.arange(SLAB)] = 1.0
        PK1c[:, _S128:_S128 + SLAB] = S128m
        B4m = np.zeros((SLAB, JB), f32)
        B4m[:, bsel] = 1.0
        PK1c[0:SLAB, _B4:_B4 + JB] = B4m

        in_maps.append(dict(PKB=PKBc, PK1=PK1c))

    # cheap host check: no real pair beyond CUTOFF (mask was dropped)
    dmax2 = ((np.abs(pos_b).max(0) + np.abs(y_all).max((0, 1))) ** 2).sum()
    if dmax2 > CUTOFF * CUTOFF:
        d = pos_b[None, :, None, :] - pos_b[:, None, None, :] + \
            shifts[None, None, :, :]
        if (d * d).sum(-1).max() > CUTOFF * CUTOFF:
            return None
    return in_maps


def _numpy_fallback(Z, pos, shift_int, cell, c6ab, r0ab, rcov, r2r4):
    f32 = np.float32
    Zi = np.asarray(Z).astype(np.int64)
    pos_b = np.asarray(pos, f32) / f32(AUTOANG)
    cell_b = np.asarray(cell, f32) / f32(AUTOANG)
    shifts = np.asarray(shift_int, f32) @ cell_b
    d = pos_b[None, :, None, :] - pos_b[:, None, None, :] + shifts[None, None, :, :]
    r2 = (d * d).sum(-1)
    mask = r2 > 1e-8
    r = np.sqrt(np.where(mask, r2, 1.0))
    in_cut = mask & (r <= CUTOFF)
    rcov_z = np.asarray(rcov, f32)[Zi]
    rco = rcov_z[:, None] + rcov_z[None, :]
    dmp = 1.0 / (1.0 + np.exp(-K1 * (rco[:, :, None] / r - 1.0)))
    cn = np.where(mask & (r <= CNTHR), dmp, 0.0).sum(axis=(1, 2))
    tbl = np.asarray(c6ab, f32)[Zi[:, None], Zi[None, :]]
    c6r = tbl[..., 0]
    valid = c6r > 0.0
    dcn = (cn[:, None, None, None] - tbl[..., 1]) ** 2 + \
          (cn[None, :, None, None] - tbl[..., 2]) ** 2
    dmin = np.where(valid, dcn, 1e10).min(axis=(-2, -1), keepdims=True)
    w = np.where(valid, np.exp(K3 * (dcn - dmin)), 0.0)
    c6 = (c6r * w).sum((-2, -1)) / np.maximum(w.sum((-2, -1)), 1e-20)
    r2r4_z = np.asarray(r2r4, f32)[Zi]
    c8 = 3.0 * c6 * r2r4_z[:, None] * r2r4_z[None, :]
    r0 = np.asarray(r0ab, f32)[Zi[:, None], Zi[None, :]]
    r6 = np.where(mask, r2, 1.0) ** 3
    r8 = r6 * np.where(mask, r2, 1.0)
    t6 = (r / (RS6 * r0[:, :, None])) ** (-ALP)
    t8 = (r / (RS18 * r0[:, :, None])) ** (-(ALP + 2.0))
    e6 = S6 * c6[:, :, None] / r6 / (1.0 + 6.0 * t6)
    e8 = S18 * c8[:, :, None] / r8 / (1.0 + 6.0 * t8)
    E = -0.5 * np.where(in_cut, e6 + e8, 0.0).sum(dtype=np.float64)
    return np.asarray(np.float32(AUTOEV * E))


_PROGRAM_CACHE = {}


def kernel(**inputs) -> np.ndarray:
    inputs = {k: np.asarray(v) for k, v in inputs.items()}
    shapes_ok = (inputs["pos"].shape == (N, 3)
                 and inputs["shift_int"].shape == (NS, 3)
                 and inputs["c6ab"].shape == (NZ, NZ, NG, NG, 3))
    in_maps = _host_prep(**inputs) if shapes_ok else None
    if in_maps is None:
        return _numpy_fallback(**inputs)

    if "nc" not in _PROGRAM_CACHE:
        _PROGRAM_CACHE["nc"] = build_program()
    nc = _PROGRAM_CACHE["nc"]

    trace = bool(os.environ.get("D3_TRACE"))
    res = run_bass_kernel_spmd(nc, in_maps, list(range(NCORES)), trace=trace)
    _PROGRAM_CACHE["last_exec_time_ns"] = res.exec_time_ns
    _PROGRAM_CACHE["last_results"] = res
    e = np.zeros((), np.float64)
    for c in range(NCORES):
        e += res.results[c]["e_out"].astype(np.float64).sum()
    out = np.float32(-0.5 * AUTOEV * S6 * e)
    return np.asarray(out)


if __name__ == "__main__":
    nc = build_program()
    print("program built:",
          sum(len(b.instructions) for f in nc.m.functions for b in f.blocks),
          "instructions")


# revision 11
# speedup vs baseline: 1.0884x; 1.0273x over previous
"""DFT-D3 (zero damping, static all-pairs) two-body dispersion energy on 8
Trainium2 NeuronCores — v2 (optimized schedule).

Structure (i-slab sharding, 64 atoms per core):
  - r2[j,(i,s)] for the slab via one bf16 TensorE matmul chain per 128-j
    block (3-way bf16-split operands ~24-bit; self-pair offset rows merged
    into the matmul; tiny eps folded into |y|^2 so no clamp pass is needed).
  - Log-space rationals: L2 = ln r2 (ScalarE reads PSUM directly);
    damping factors are sigmoids of affine functions of L2.
  - CN path: damp = sigmoid(K1*(rco*e^{-L2/2} - 1)); the sigmoid's
    accum_out produces the full-tile per-partition sum = this core's
    contribution to cn[j] for the 128 j's of the block (valid because the
    shift set is symmetric), giving a [128,4] partial vector -> one
    AllReduce(add). No reduce/transpose matmuls needed.
  - Slab extraction from the reduced cn (per-core data, same program):
    cn_col[i] = reduce_b(B4 * (S128^T @ cn_t)) with one-hot S128/B4 inputs.
  - A/B chains: g6/g8 computed on GpSimd (scalar_tensor_tensor), u6/u8 and
    sigmoids on ScalarE (batched by activation-table set: log/sig/log/sig),
    products in bf16 (2x DVE mode), merged [128, 2*1728] tiles.
  - Phase 2 (c6 interpolation) exploits the separable c6ab grid structure;
    phi chains fused into [128,20] / [64,5] tiles; E reduced to a scalar
    on-device (one-descriptor output DMA).
"""
import os
os.environ.setdefault("JAX_PLATFORMS", "cpu")

import math
import numpy as np
import ml_dtypes

import concourse.bass as bass
import concourse.mybir as mybir
from concourse.tile import TileContext, add_dep_helper
from concourse.bass_utils import run_bass_kernel_spmd

F32 = mybir.dt.float32
BF16 = mybir.dt.bfloat16
AF = mybir.ActivationFunctionType
OP = mybir.AluOpType

# D3 constants
AUTOANG = 0.52917726
AUTOEV = 27.21138505
K1, K3 = 16.0, -4.0
CUTOFF, CNTHR = 95.0, 40.0
S6, RS6, S18, RS18, ALP = 1.0, 1.217, 0.722, 1.0, 14.0

N = 512
NS = 27
NCORES = 8
SLAB = N // NCORES           # 64
JB = 4                       # j blocks of 128
FREE = SLAB * NS             # 1728
HALF = FREE // 2             # 864
NZ = 95
NG = 5

SELF_R2 = 1.0e4
R2EPS = 1.0e-4
C14L = float(math.log(6.0) + 14.0 * math.log(RS6))
C16L = float(math.log(6.0) + 16.0 * math.log(RS18))
CB8 = float(3.0 * S18)

# f32 pack PK1 [128, C1] column offsets
_OHZT = 0                      # [95, 512]
_LRCO = 512                    # [2, 512]
_R0ABT = 1024                  # [95, 95]
_OHZIT = 1119                  # [95, 64]
_RRCO = 1183                   # [2, 64]
_R2R4I = 1247                  # [128, 64]
_R2R4C = 1311                  # [128, 4]
_G20 = 1315                    # [128, 20]
_OHZ = 1335                    # [128, 380] (4 x 95)
_S128 = 1715                   # [128, 64]
_B4 = 1779                     # [64, 4]
_I64 = 1783                    # [64, 64]
_G5 = 1847                     # [64, 5]
C1 = 1852

# bf16 pack PKB [95, CB] column offsets
_LCAT = 0                      # [94, 512]
_RCAT = 512                    # [94, 1728]
_C2H = 2240                    # [95, 5*475]
_C2L = 4615                    # [95, 5*475]
CB = 6990


def _bc_s(ap2d, s=NS):
    """[128, M] AP -> [128, M, s] with stride-0 broadcast over s."""
    a3 = ap2d[:, :, None]
    new = [list(a3.ap[0]), list(a3.ap[1]), [0, s]]
    return bass.AP(a3.tensor, a3.offset, new)


def _bc_mid(ap2d, k):
    """[P, M] AP -> 3D [P, k(bcast stride-0), M]."""
    return bass.AP(ap2d.tensor, ap2d.offset,
                   [list(ap2d.ap[0]), [0, k], list(ap2d.ap[1])])


def _split_excess_waits(nc, max_waits=1):
    """This walrus build accepts at most one sync wait per instruction;
    Tile's tail drain can carry several. Hoist excess waits onto inserted
    drains on the same engine (sequential waits == conjunction)."""
    n_split = 0
    for f in nc.m.functions:
        for b in f.blocks:
            new_list = []
            changed = False
            for ins in b.instructions:
                si = ins.sync_info
                if si is not None:
                    waits = list(si.on_wait or [])
                    updates = list(si.on_update or [])
                    if len(waits) > max_waits:
                        excess = waits[: len(waits) - max_waits]
                        keep = waits[len(waits) - max_waits:]
                        for w in excess:
                            d = mybir.InstDrain(
                                name=f"I-waitsplit-{n_split}", ins=[], outs=[])
                            n_split += 1
                            d.engine = ins.engine
                            d.sync_info = mybir.SyncInfo(on_wait=[w], on_update=[])
                            new_list.append(d)
                            changed = True
                        ins.sync_info = mybir.SyncInfo(
                            on_wait=list(keep), on_update=list(updates))
                new_list.append(ins)
            if changed:
                b.instructions = new_list
    return n_split


_orig_clear_sems = bass.Bass.clear_and_free_semaphores


def _chunked_clear_sems(self, sems, _chunk=4):
    nums = sorted(s.num if hasattr(s, "num") else s for s in sems)
    for i in range(0, len(nums), _chunk):
        _orig_clear_sems(self, nums[i:i + _chunk])


bass.Bass.clear_and_free_semaphores = _chunked_clear_sems


def build_program():
    nc = bass.Bass(num_devices=NCORES)

    PKB = nc.dram_tensor("PKB", [NZ, CB], BF16, kind="ExternalInput")
    PK1 = nc.dram_tensor("PK1", [128, C1], F32, kind="ExternalInput")
    e_out = nc.dram_tensor("e_out", [1], F32, kind="ExternalOutput")

    with TileContext(nc) as tc:
        with (
            tc.tile_pool(name="const", bufs=1) as cpool,
            tc.tile_pool(name="L2p", bufs=1) as l2pool,
            tc.tile_pool(name="usp", bufs=4) as uspool,
            tc.tile_pool(name="up", bufs=1) as upool,
            tc.tile_pool(name="gp", bufs=3) as gpool,
            tc.tile_pool(name="recp", bufs=2) as recpool,
            tc.tile_pool(name="mp", bufs=2) as mpool,
            tc.tile_pool(name="sm", bufs=3) as spool,
            tc.tile_pool(name="ph2", bufs=1) as ppool,
            tc.tile_pool(name="psA", bufs=2, space="PSUM") as psA,
            tc.tile_pool(name="psS", bufs=2, space="PSUM") as psS,
            tc.tile_pool(name="psW", bufs=1, space="PSUM") as psW,
            tc.tile_pool(name="psP", bufs=1, space="PSUM") as psP,
            tc.tile_pool(name="dram", bufs=1, space="DRAM") as dpool,
        ):
            # ---------- input DMAs: 2 packed tensors, split across queues
            pkb_s = cpool.tile([NZ, CB], BF16, tag="pkb")
            # critical first: Lcat+Rcat columns (phase-1 matmul operands)
            nc.sync.dma_start(pkb_s[:, 0:1376], PKB[:, 0:1376])
            nc.scalar.dma_start(pkb_s[:, 1376:_C2H], PKB[:, 1376:_C2H])
            pk1_s = cpool.tile([128, C1], F32, tag="pk1")
            nc.gpsimd.dma_start(pk1_s[:], PK1[:])
            # phase-2-only tables later, lower priority
            nc.sync.dma_start(pkb_s[:, _C2H:], PKB[:, _C2H:])

            # slices of the packs
            Lcat = pkb_s[0:94, _LCAT:_LCAT + N]
            Rcat = pkb_s[0:94, _RCAT:_RCAT + FREE]
            ohZT = pk1_s[0:NZ, _OHZT:_OHZT + N]
            Lrco = pk1_s[0:2, _LRCO:_LRCO + N]
            r0abT = pk1_s[0:NZ, _R0ABT:_R0ABT + NZ]
            ohZiT = pk1_s[0:NZ, _OHZIT:_OHZIT + SLAB]
            Rrco = pk1_s[0:2, _RRCO:_RRCO + SLAB]
            r2r4i = pk1_s[:, _R2R4I:_R2R4I + SLAB]
            r2r4c = pk1_s[:, _R2R4C:_R2R4C + JB]
            g20 = pk1_s[:, _G20:_G20 + JB * NG]
            S128 = pk1_s[:, _S128:_S128 + SLAB]
            B4 = pk1_s[0:SLAB, _B4:_B4 + JB]
            I64 = pk1_s[0:SLAB, _I64:_I64 + SLAB]
            G5 = pk1_s[0:SLAB, _G5:_G5 + NG]

            ones1x95 = cpool.tile([1, NZ], F32, tag="ones95")
            nc.gpsimd.memset(ones1x95[:], 1.0)
            ones64 = cpool.tile([SLAB, 1], F32, tag="ones64")
            nc.gpsimd.memset(ones64[:], 1.0)
            negk1 = cpool.tile([128, 1], F32, tag="negk1")
            nc.gpsimd.memset(negk1[:], -K1)

            # ---------- prep matmuls ----------
            # R1[z1, i] = r0ab[z1, Z_i]
            R1_ps = psS.tile([NZ, SLAB], F32, tag="small")
            nc.tensor.matmul(R1_ps[:], r0abT, ohZiT, start=True, stop=True)
            R1_s = cpool.tile([NZ, SLAB], F32, tag="R1")
            nc.vector.tensor_copy(R1_s[:], R1_ps[:])

            log_batch1 = []  # all natural_log_exp-set ops of round 1
            lnc14_s, lnc16_s, rco_s = [], [], []
            for b in range(JB):
                jsl = slice(b * 128, (b + 1) * 128)
                r0p_ps = psS.tile([128, SLAB], F32, tag="small")
                nc.tensor.matmul(r0p_ps[:], ohZT[:, jsl], R1_s[:],
                                 start=True, stop=True)
                lr = spool.tile([128, SLAB], F32, tag="lnr0")
                li = nc.scalar.activation(lr[:], r0p_ps[:], AF.Ln)
                log_batch1.append(li)
                lnc14 = cpool.tile([128, SLAB], F32, tag=f"lnc14_{b}")
                nc.vector.tensor_scalar(lnc14[:], lr[:], 14.0, C14L,
                                        OP.mult, OP.add)
                lnc16 = cpool.tile([128, SLAB], F32, tag=f"lnc16_{b}")
                nc.vector.tensor_scalar(lnc16[:], lr[:], 16.0, C16L,
                                        OP.mult, OP.add)
                lnc14_s.append(lnc14)
                lnc16_s.append(lnc16)

                rco_ps = psS.tile([128, SLAB], F32, tag="small")
                nc.tensor.matmul(rco_ps[:], Lrco[:, jsl], Rrco,
                                 start=True, stop=True)
                rco = cpool.tile([128, SLAB], F32, tag=f"rco_{b}")
                nc.vector.tensor_copy(rco[:], rco_ps[:])
                rco_s.append(rco)

            # ---------- phase 1a: r2, L2, CN partials ----------
            cnpart = ppool.tile([128, JB], F32, tag="cnpart")
            L2_s, g_s = [], []
            sigcn_insts = []
            for b in range(JB):
                jsl = slice(b * 128, (b + 1) * 128)
                L2 = l2pool.tile([128, FREE], F32, tag=f"L2_{b}")
                L2_s.append(L2)
                for h in range(2):
                    r2_ps = psA.tile([128, HALF], F32, tag="r2ps")
                    c0 = h * HALF
                    nc.tensor.matmul(r2_ps[:, 0:512], Lcat[:, jsl],
                                     Rcat[:, c0:c0 + 512],
                                     start=True, stop=True)
                    nc.tensor.matmul(r2_ps[:, 512:HALF], Lcat[:, jsl],
                                     Rcat[:, c0 + 512:c0 + HALF],
                                     start=True, stop=True)
                    li = nc.scalar.activation(L2[:, c0:c0 + HALF], r2_ps[:],
                                              AF.Ln)
                    log_batch1.append(li)
                # g6 on V as soon as L2(b) exists; g8 later (its half of
                # the g tile doubles as the cn-path scratch first)
                g = gpool.tile([128, 2 * FREE], F32, tag=f"g68_{b}")
                g_s.append(g)
                nc.vector.scalar_tensor_tensor(
                    g[:, 0:FREE].rearrange("p (i s) -> p i s", s=NS),
                    L2[:].rearrange("p (i s) -> p i s", s=NS),
                    float(ALP / 2.0), _bc_s(lnc14_s[b][:]),
                    OP.mult, OP.subtract)
                # cn chain in the g8 half: us = e^{-L2/2}; *rco; sigmoid+accum
                us = g[:, FREE:2 * FREE]
                ei = nc.scalar.activation(us, L2[:], AF.Exp, scale=-0.5)
                log_batch1.append(ei)
                nc.vector.tensor_tensor(
                    us.rearrange("p (i s) -> p i s", s=NS),
                    us.rearrange("p (i s) -> p i s", s=NS),
                    _bc_s(rco_s[b][:]), OP.mult)
                si = nc.scalar.activation(us, us, AF.Sigmoid,
                                          bias=negk1[:], scale=K1,
                                          accum_out=cnpart[:, b:b + 1])
                sigcn_insts.append(si)

            for si in sigcn_insts:
                add_dep_helper(si.ins, log_batch1[-1].ins, sync=False,
                               reason="sigmoid set after log set (round 1)")
                add_dep_helper(si.ins, log_batch1[3].ins, sync=False,
                               reason="after prep lnr0 Ln ops too")
            for b in range(JB):
                nc.vector.scalar_tensor_tensor(
                    g_s[b][:, FREE:2 * FREE].rearrange("p (i s) -> p i s", s=NS),
                    L2_s[b][:].rearrange("p (i s) -> p i s", s=NS),
                    float((ALP + 2.0) / 2.0), _bc_s(lnc16_s[b][:]),
                    OP.mult, OP.subtract)

            # ---------- CN collective (AllReduce of [128,4] partials) ----
            cc_in = dpool.tile([128, JB], F32, tag="ccin")
            cc_out = dpool.tile([NCORES * 128, JB], F32, tag="ccout")
            nc.gpsimd.dma_start(cc_in[:], cnpart[:])
            nc.gpsimd.collective_compute(
                "AllGather", OP.bypass, replica_groups=[list(range(NCORES))],
                ins=[cc_in.opt()], outs=[cc_out.opt()],
            )
            cn8 = ppool.tile([128, NCORES * JB], F32, tag="cn8")
            _qs = [nc.gpsimd, nc.sync, nc.scalar]
            for c in range(NCORES):
                _qs[c % 3].dma_start(cn8[:, c * JB:(c + 1) * JB],
                                     cc_out[c * 128:(c + 1) * 128, :])
            cn_t = ppool.tile([128, JB], F32, tag="cn_t")
            cn8v = cn8[:, :]
            cn8_3d = bass.AP(cn8v.tensor, cn8v.offset,
                             [list(cn8v.ap[0]), [1, JB], [JB, NCORES]])
            nc.vector.tensor_reduce(cn_t[:], cn8_3d,
                                    axis=mybir.AxisListType.X, op=OP.add)

            # ---------- phase 1b: u6/u8 exps, sigmoids, products, reduces
            e_batch2 = []
            u_s = []
            for b in range(JB):
                u = upool.tile([128, 2 * FREE], BF16, tag=f"u68_{b}")
                u_s.append(u)
                e3 = nc.scalar.activation(u[:, 0:FREE], L2_s[b][:],
                                          AF.Exp, scale=-3.0)
                e4 = nc.scalar.activation(u[:, FREE:2 * FREE], L2_s[b][:],
                                          AF.Exp, scale=-4.0)
                add_dep_helper(e3.ins, sigcn_insts[-1].ins, sync=False,
                               reason="log set round 2 after cn sigmoids")
                add_dep_helper(e4.ins, sigcn_insts[-1].ins, sync=False,
                               reason="log set round 2 after cn sigmoids")
                e_batch2 += [e3, e4]

            AB_s = []
            last_sig68 = None
            for b in range(JB):
                rec = recpool.tile([128, 2 * FREE], BF16, tag="rec")
                s6i = nc.scalar.activation(rec[:, 0:FREE], g_s[b][:, 0:FREE],
                                           AF.Sigmoid, scale=float(ALP / 2.0))
                s8i = nc.scalar.activation(rec[:, FREE:2 * FREE],
                                           g_s[b][:, FREE:2 * FREE],
                                           AF.Sigmoid,
                                           scale=float((ALP + 2.0) / 2.0))
                add_dep_helper(s6i.ins, e_batch2[-1].ins, sync=False,
                               reason="sigmoid set round 2 after exps")
                add_dep_helper(s8i.ins, e_batch2[-1].ins, sync=False,
                               reason="sigmoid set round 2 after exps")
                last_sig68 = s8i
                m = mpool.tile([128, 2 * FREE], BF16, tag="m68")
                nc.vector.tensor_tensor(m[:], u_s[b][:], rec[:], OP.mult)
                AB = ppool.tile([128, 2 * SLAB], F32, tag=f"AB_{b}")
                nc.vector.tensor_reduce(
                    AB[:], m[:].rearrange("p (k s) -> p k s", s=NS),
                    axis=mybir.AxisListType.X, op=OP.add)
                AB_s.append(AB)

            # ---------- phase 2: phi for all atoms ([128, (a,b)=20]) ----
            d20 = ppool.tile([128, JB * NG], F32, tag="d20")
            nc.vector.tensor_tensor(
                d20[:].rearrange("p (a b) -> p a b", a=NG),
                _bc_mid(cn_t[:], NG), g20.rearrange("p (a b) -> p a b", a=NG),
                OP.subtract)
            sq20 = ppool.tile([128, JB * NG], F32, tag="sq20")
            nc.vector.tensor_tensor(sq20[:], d20[:], d20[:], OP.mult)
            mn4 = ppool.tile([128, JB], F32, tag="mn4")
            nc.vector.tensor_reduce(
                mn4[:], sq20[:].rearrange("p (a b) -> p b a", a=NG),
                axis=mybir.AxisListType.X, op=OP.min)
            dt20 = ppool.tile([128, JB * NG], F32, tag="dt20")
            nc.vector.tensor_tensor(
                dt20[:].rearrange("p (a b) -> p a b", a=NG),
                sq20[:].rearrange("p (a b) -> p a b", a=NG),
                _bc_mid(mn4[:], NG), OP.subtract)
            ex20 = ppool.tile([128, JB * NG], F32, tag="ex20")
            xi = nc.scalar.activation(ex20[:], dt20[:], AF.Exp, scale=K3)
            add_dep_helper(xi.ins, last_sig68.ins, sync=False,
                           reason="log set round 3 after A/B sigmoids")
            ss4 = ppool.tile([128, JB], F32, tag="ss4")
            nc.vector.tensor_reduce(
                ss4[:], ex20[:].rearrange("p (a b) -> p b a", a=NG),
                axis=mybir.AxisListType.X, op=OP.add)
            rs4 = ppool.tile([128, JB], F32, tag="rs4")
            nc.vector.reciprocal(rs4[:], ss4[:])
            phi20 = ppool.tile([128, JB * NG], F32, tag="phi20")
            nc.vector.tensor_tensor(
                phi20[:].rearrange("p (a b) -> p a b", a=NG),
                ex20[:].rearrange("p (a b) -> p a b", a=NG),
                _bc_mid(rs4[:], NG), OP.mult)

            # ---------- slab cn + phi^T ([64, 5]-land) ----------
            csel_ps = psS.tile([SLAB, JB], F32, tag="small")
            nc.tensor.matmul(csel_ps[:], S128, cn_t[:], start=True, stop=True)
            csel = ppool.tile([SLAB, JB], F32, tag="csel")
            nc.vector.tensor_tensor(csel[:], csel_ps[:], B4, OP.mult)
            cn_col = ppool.tile([SLAB, 1], F32, tag="cncol")
            nc.vector.tensor_reduce(cn_col[:], csel[:],
                                    axis=mybir.AxisListType.X, op=OP.add)
            dP = ppool.tile([SLAB, NG], F32, tag="dP")
            cnb = cn_col[:, :]
            cnb3 = bass.AP(cnb.tensor, cnb.offset,
                           [list(cnb.ap[0]), [0, NG]])
            nc.vector.tensor_tensor(dP[:], cnb3, G5, OP.subtract)
            sqP = ppool.tile([SLAB, NG], F32, tag="sqP")
            nc.vector.tensor_tensor(sqP[:], dP[:], dP[:], OP.mult)
            mnP = ppool.tile([SLAB, 1], F32, tag="mnP")
            nc.vector.tensor_reduce(mnP[:], sqP[:],
                                    axis=mybir.AxisListType.X, op=OP.min)
            mnP3 = bass.AP(mnP[:, :].tensor, mnP[:, :].offset,
                           [list(mnP[:, :].ap[0]), [0, NG]])
            dtP = ppool.tile([SLAB, NG], F32, tag="dtP")
            nc.vector.tensor_tensor(dtP[:], sqP[:], mnP3, OP.subtract)
            exP = ppool.tile([SLAB, NG], F32, tag="exP")
            xi2 = nc.scalar.activation(exP[:], dtP[:], AF.Exp, scale=K3)
            add_dep_helper(xi2.ins, last_sig68.ins, sync=False,
                           reason="log set round 3 after A/B sigmoids")
            sP = ppool.tile([SLAB, 1], F32, tag="sP")
            nc.vector.tensor_reduce(sP[:], exP[:],
                                    axis=mybir.AxisListType.X, op=OP.add)
            rP = ppool.tile([SLAB, 1], F32, tag="rP")
            nc.vector.reciprocal(rP[:], sP[:])
            rP3 = bass.AP(rP[:, :].tensor, rP[:, :].offset,
                          [list(rP[:, :].ap[0]), [0, NG]])
            phiP = ppool.tile([SLAB, NG], F32, tag="phiP")
            nc.vector.tensor_tensor(phiP[:], exP[:], rP3, OP.mult)
            # transpose phiP -> [5, 64] via matmul with identity
            phiT_ps = psS.tile([NG, SLAB], F32, tag="small")
            nc.tensor.matmul(phiT_ps[:], phiP[:], I64, start=True, stop=True)
            phiT5 = ppool.tile([NG, SLAB], F32, tag="phiT5")
            nc.vector.tensor_copy(phiT5[:], phiT_ps[:])
            phiT_a = []
            for a in range(NG):
                pt = ppool.tile([1, SLAB], F32, tag=f"phiT_{a}")
                nc.gpsimd.dma_start(pt[:], phiT5[a:a + 1, :])
                phiT_a.append(pt)

            # ---------- phase 2: Q, G, contractions ----------
            W2_ps = psW.tile([SLAB, NZ * NG], F32, tag="W2")
            for b in range(JB):
                Q = spool.tile([128, NZ * NG], BF16, tag="Q")
                for a in range(NG):
                    nc.vector.tensor_scalar(
                        Q[:, a * NZ:(a + 1) * NZ],
                        pk1_s[:, _OHZ + b * NZ:_OHZ + (b + 1) * NZ],
                        phi20[:, a * JB + b:a * JB + b + 1], None, OP.mult)
                t1 = spool.tile([128, SLAB], F32, tag="g_t1")
                nc.vector.tensor_scalar(t1[:], AB_s[b][:, SLAB:2 * SLAB],
                                        r2r4c[:, b:b + 1], None, OP.mult)
                t2 = spool.tile([128, SLAB], F32, tag="g_t2")
                nc.vector.tensor_tensor(t2[:], t1[:], r2r4i, OP.mult)
                G = spool.tile([128, SLAB], BF16, tag="G")
                nc.vector.scalar_tensor_tensor(
                    G[:], t2[:], CB8, AB_s[b][:, 0:SLAB], OP.mult, OP.add)
                nc.tensor.matmul(W2_ps[:], G[:], Q[:],
                                 start=(b == 0), stop=(b == JB - 1))

            PC_ps = psP.tile([SLAB, NZ * NG], F32, tag="PC")
            for a in range(NG):
                phiA_ps = psS.tile([NZ, SLAB], F32, tag="small")
                nc.tensor.matmul(phiA_ps[:], ones1x95[:],
                                 phiT_a[a][:], start=True, stop=True)
                PT = spool.tile([NZ, SLAB], BF16, tag="PT")
                nc.vector.tensor_tensor(PT[:], ohZiT, phiA_ps[:], OP.mult)
                nc.tensor.matmul(PC_ps[:], PT[:],
                                 pkb_s[:, _C2H + a * NZ * NG:
                                       _C2H + (a + 1) * NZ * NG],
                                 start=(a == 0), stop=False)
                nc.tensor.matmul(PC_ps[:], PT[:],
                                 pkb_s[:, _C2L + a * NZ * NG:
                                       _C2L + (a + 1) * NZ * NG],
                                 start=False, stop=(a == NG - 1))
            PC_s = spool.tile([SLAB, NZ * NG], F32, tag="PCs")
            nc.vector.tensor_copy(PC_s[:], PC_ps[:])

            scr = spool.tile([SLAB, NZ * NG], F32, tag="scr")
            nc.vector.tensor_tensor(scr[:], W2_ps[:], PC_s[:], OP.mult)
            E_col = ppool.tile([SLAB, 1], F32, tag="Ecol")
            nc.vector.tensor_reduce(E_col[:], scr[:],
                                    axis=mybir.AxisListType.X, op=OP.add)
            E_ps = psS.tile([1, 1], F32, tag="small")
            nc.tensor.matmul(E_ps[:], ones64[:], E_col[:],
                             start=True, stop=True)
            E_s = ppool.tile([1, 1], F32, tag="Es")
            nc.scalar.copy(E_s[:], E_ps[:])
            nc.sync.dma_start(e_out[:], E_s[0, :])

    _split_excess_waits(nc)
    return nc


# ----------------------------------------------------------------------
# host side
# ----------------------------------------------------------------------

def _check_separable(c6ab):
    t1 = c6ab[..., 1]
    t2 = c6ab[..., 2]
    g = t1[0, 0, :, 0]
    ok = (np.abs(t1 - g[None, None, :, None]).max() == 0.0
          and np.abs(t2 - g[None, None, None, :]).max() == 0.0
          and (c6ab[..., 0] > 0).all())
    return ok, g.astype(np.float32)


def _host_prep(Z, pos, shift_int, cell, c6ab, r0ab, rcov, r2r4):
    f32 = np.float32
    Zi = np.clip(np.asarray(Z).astype(np.int64), 0, NZ - 1)
    pos_b = (np.asarray(pos, f32) / f32(AUTOANG)).astype(f32)
    cell_b = (np.asarray(cell, f32) / f32(AUTOANG)).astype(f32)
    shifts = (np.asarray(shift_int, f32) @ cell_b).astype(f32)
    rcov_z = np.asarray(rcov, f32)[Zi]
    r2r4_z = np.asarray(r2r4, f32)[Zi]

    # the shift-set symmetry (s -> -s closure) is required for the
    # AllReduce cn formulation; verify cheaply
    sh = np.asarray(shift_int, np.int64)
    sset = set(map(tuple, sh.tolist()))
    if not all(tuple(-np.array(t)) in sset for t in sset):
        return None

    ok, g = _check_separable(np.asarray(c6ab, f32))
    if not ok:
        return None

    bf16 = ml_dtypes.bfloat16

    def split3(x):
        x = np.asarray(x, np.float64)
        h = x.astype(bf16)
        r = x - h.astype(np.float64)
        m = r.astype(bf16)
        l = (r - m.astype(np.float64)).astype(bf16)
        return h, m, l

    # ---- bf16 pack: Lcat / Rcat / C2 tables ----
    pj2 = (pos_b.astype(np.float64) ** 2).sum(-1)
    Lcat = np.zeros((94, N), bf16)
    ph, pm, pl = split3(pos_b.T)
    p2h, p2m, p2l = split3(pj2)
    onesN = np.ones(N, bf16)
    for d in range(3):
        base = d * 8
        Lcat[base + 0] = ph[d]; Lcat[base + 1] = ph[d]
        Lcat[base + 2] = pm[d]; Lcat[base + 3] = pm[d]
        Lcat[base + 4] = ph[d]; Lcat[base + 5] = pl[d]
        Lcat[base + 6] = pm[d]; Lcat[base + 7] = pl[d]
    Lcat[24] = p2h; Lcat[25] = p2m; Lcat[26] = p2l
    Lcat[27] = onesN; Lcat[28] = onesN; Lcat[29] = onesN

    C2 = np.asarray(c6ab, np.float64)[..., 0].transpose(2, 0, 3, 1).reshape(
        NZ * NG, NZ * NG)
    # reorder rows (a, z) -> [z, (a, z2g)] layout: row z holds a-blocks
    C2h_f = C2.astype(bf16)
    C2l_f = (C2 - C2h_f.astype(np.float64)).astype(bf16)
    # [NZ*NG, NZ*NG] with row index (a*NZ + z) -> [NZ, NG*NZ*NG]
    C2h_r = np.ascontiguousarray(
        C2h_f.reshape(NG, NZ, NZ * NG).transpose(1, 0, 2).reshape(NZ, NG * NZ * NG))
    C2l_r = np.ascontiguousarray(
        C2l_f.reshape(NG, NZ, NZ * NG).transpose(1, 0, 2).reshape(NZ, NG * NZ * NG))

    oh = np.zeros((N, NZ), f32)
    oh[np.arange(N), Zi] = 1.0
    r0s = np.asarray(r0ab, f32)

    # ---- shared part of f32 pack ----
    PK1_shared = np.zeros((128, C1), f32)
    PK1_shared[0:NZ, _OHZT:_OHZT + N] = oh.T
    PK1_shared[0:2, _LRCO:_LRCO + N] = np.stack(
        [rcov_z, np.ones(N, f32)], axis=0)
    PK1_shared[0:NZ, _R0ABT:_R0ABT + NZ] = r0s.T
    PK1_shared[:, _R2R4C:_R2R4C + JB] = r2r4_z.reshape(JB, 128).T
    PK1_shared[:, _G20:_G20 + JB * NG] = np.repeat(g, JB)[None, :]
    PK1_shared[:, _OHZ:_OHZ + JB * NZ] = np.concatenate(
        [oh[b * 128:(b + 1) * 128, :] for b in range(JB)], axis=1)
    PK1_shared[0:SLAB, _I64:_I64 + SLAB] = np.eye(SLAB, dtype=f32)
    PK1_shared[0:SLAB, _G5:_G5 + NG] = np.broadcast_to(g[None, :], (SLAB, NG))

    y_all = pos_b[:, None, :] - shifts[None, :, :]          # [N, S, 3]
    in_maps = []
    for c in range(NCORES):
        isl = slice(c * SLAB, (c + 1) * SLAB)
        y = y_all[isl].reshape(FREE, 3).astype(f32)
        y2 = (y.astype(np.float64) ** 2).sum(-1) + R2EPS
        q = -2.0 * y.astype(np.float64)
        Rcat = np.zeros((94, FREE), bf16)
        qh, qm, ql = split3(q.T)
        y2h, y2m, y2l = split3(y2)
        onesF = np.ones(FREE, bf16)
        for d in range(3):
            base = d * 8
            Rcat[base + 0] = qh[d]; Rcat[base + 1] = qm[d]
            Rcat[base + 2] = qh[d]; Rcat[base + 3] = qm[d]
            Rcat[base + 4] = ql[d]; Rcat[base + 5] = qh[d]
            Rcat[base + 6] = ql[d]; Rcat[base + 7] = qm[d]
        Rcat[24] = onesF; Rcat[25] = onesF; Rcat[26] = onesF
        Rcat[27] = y2h; Rcat[28] = y2m; Rcat[29] = y2l
        Lc = Lcat.copy()
        Lc[30 + np.arange(SLAB), c * SLAB + np.arange(SLAB)] = bf16(SELF_R2)
        Rcat[30 + np.arange(SLAB), np.arange(SLAB) * NS + (NS // 2)] = bf16(1.0)

        PKBc = np.zeros((NZ, CB), bf16)
        PKBc[0:94, _LCAT:_LCAT + N] = Lc
        PKBc[0:94, _RCAT:_RCAT + FREE] = Rcat
        PKBc[:, _C2H:_C2H + NG * NZ * NG] = C2h_r
        PKBc[:, _C2L:_C2L + NG * NZ * NG] = C2l_r

        PK1c = PK1_shared.copy()
        PK1c[0:NZ, _OHZIT:_OHZIT + SLAB] = oh[isl].T
        PK1c[0:2, _RRCO:_RRCO + SLAB] = np.stack(
            [np.ones(SLAB, f32), rcov_z[isl]], axis=0)
        PK1c[:, _R2R4I:_R2R4I + SLAB] = np.broadcast_to(
            r2r4_z[isl][None, :], (128, SLAB))
        p0 = 64 * (c % 2)
        bsel = c // 2
        S128m = np.zeros((128, SLAB), f32)
        S128m[p0 + np.arange(SLAB), np.arange(SLAB)] = 1.0
        PK1c[:, _S128:_S128 + SLAB] = S128m
        B4m = np.zeros((SLAB, JB), f32)
        B4m[:, bsel] = 1.0
        PK1c[0:SLAB, _B4:_B4 + JB] = B4m

        in_maps.append(dict(PKB=PKBc, PK1=PK1c))

    # cheap host check: no real pair beyond CUTOFF (mask was dropped)
    dmax2 = ((np.abs(pos_b).max(0) + np.abs(y_all).max((0, 1))) ** 2).sum()
    if dmax2 > CUTOFF * CUTOFF:
        d = pos_b[None, :, None, :] - pos_b[:, None, None, :] + \
            shifts[None, None, :, :]
        if (d * d).sum(-1).max() > CUTOFF * CUTOFF:
            return None
    return in_maps


def _numpy_fallback(Z, pos, shift_int, cell, c6ab, r0ab, rcov, r2r4):
    f32 = np.float32
    Zi = np.asarray(Z).astype(np.int64)
    pos_b = np.asarray(pos, f32) / f32(AUTOANG)
    cell_b = np.asarray(cell, f32) / f32(AUTOANG)
    shifts = np.asarray(shift_int, f32) @ cell_b
    d = pos_b[None, :, None, :] - pos_b[:, None, None, :] + shifts[None, None, :, :]
    r2 = (d * d).sum(-1)
    mask = r2 > 1e-8
    r = np.sqrt(np.where(mask, r2, 1.0))
    in_cut = mask & (r <= CUTOFF)
    rcov_z = np.asarray(rcov, f32)[Zi]
    rco = rcov_z[:, None] + rcov_z[None, :]
    dmp = 1.0 / (1.0 + np.exp(-K1 * (rco[:, :, None] / r - 1.0)))
    cn = np.where(mask & (r <= CNTHR), dmp, 0.0).sum(axis=(1, 2))
    tbl = np.asarray(c6ab, f32)[Zi[:, None], Zi[None, :]]
    c6r = tbl[..., 0]
    valid = c6r > 0.0
    dcn = (cn[:, None, None, None] - tbl[..., 1]) ** 2 + \
          (cn[None, :, None, None] - tbl[..., 2]) ** 2
    dmin = np.where(valid, dcn, 1e10).min(axis=(-2, -1), keepdims=True)
    w = np.where(valid, np.exp(K3 * (dcn - dmin)), 0.0)
    c6 = (c6r * w).sum((-2, -1)) / np.maximum(w.sum((-2, -1)), 1e-20)
    r2r4_z = np.asarray(r2r4, f32)[Zi]
    c8 = 3.0 * c6 * r2r4_z[:, None] * r2r4_z[None, :]
    r0 = np.asarray(r0ab, f32)[Zi[:, None], Zi[None, :]]
    r6 = np.where(mask, r2, 1.0) ** 3
    r8 = r6 * np.where(mask, r2, 1.0)
    t6 = (r / (RS6 * r0[:, :, None])) ** (-ALP)
    t8 = (r / (RS18 * r0[:, :, None])) ** (-(ALP + 2.0))
    e6 = S6 * c6[:, :, None] / r6 / (1.0 + 6.0 * t6)
    e8 = S18 * c8[:, :, None] / r8 / (1.0 + 6.0 * t8)
    E = -0.5 * np.where(in_cut, e6 + e8, 0.0).sum(dtype=np.float64)
    return np.asarray(np.float32(AUTOEV * E))


_PROGRAM_CACHE = {}


def kernel(**inputs) -> np.ndarray:
    inputs = {k: np.asarray(v) for k, v in inputs.items()}
    shapes_ok = (inputs["pos"].shape == (N, 3)
                 and inputs["shift_int"].shape == (NS, 3)
                 and inputs["c6ab"].shape == (NZ, NZ, NG, NG, 3))
    in_maps = _host_prep(**inputs) if shapes_ok else None
    if in_maps is None:
        return _numpy_fallback(**inputs)

    if "nc" not in _PROGRAM_CACHE:
        _PROGRAM_CACHE["nc"] = build_program()
    nc = _PROGRAM_CACHE["nc"]

    trace = bool(os.environ.get("D3_TRACE"))
    res = run_bass_kernel_spmd(nc, in_maps, list(range(NCORES)), trace=trace)
    _PROGRAM_CACHE["last_exec_time_ns"] = res.exec_time_ns
    _PROGRAM_CACHE["last_results"] = res
    e = np.zeros((), np.float64)
    for c in range(NCORES):
        e += res.results[c]["e_out"].astype(np.float64).sum()
    out = np.float32(-0.5 * AUTOEV * S6 * e)
    return np.asarray(out)


if __name__ == "__main__":
    nc = build_program()
    print("program built:",
          sum(len(b.instructions) for f in nc.m.functions for b in f.blocks),
          "instructions")
